# revision 14
# baseline (speedup 1.0000x reference)
"""Trainium2 Bass kernel for nn_ClusteringMultiTaskLSTM.

Self-contained: builds + runs an SPMD kernel on 8 NeuronCores.
  - Encoder: one cluster per core (C=8), 2-layer LSTM over T_IN steps,
    transposed layout (dim, B), fp32r (TF32) matmuls.
  - AllReduce of final encoder states across cores (cluster mean).
  - Decoder: 8 per-feature decoders per core (features 8c..8c+7 all belong
    to cluster c), T_OUT steps of LSTM cell + batch-axis attention + fc2.

Math tricks (exact, not approximations):
  - sigmoid(x) = (1 + tanh(x/2))/2 -> all gates use one tanh table set; the
    1/2 input scale is folded into i,f,o weight rows host-side; the output
    affine is folded into fused scalar_tensor_tensor ops via a doubled
    state: S = 2c, hS = 2h, with h-consuming weights pre-halved host-side.
  - attention scores: q.k = qkv^T (Wk^T Wq) qkv + per-key term + per-query
    terms; per-query terms are softmax-invariant and dropped; the per-key
    term becomes the exp() per-partition bias; softmax normalization is
    deferred through the (linear) attn->out_w->fc2 chain and applied as a
    single multiply by 1/colsum on the (1,B) fc2 output.
  - v bias + out_w bias + fc2 bias collapse to one scalar beta per feature.
"""
import sys, traceback
if '/opt/trn_rl_repo' not in sys.path:
    sys.path.insert(0, '/opt/trn_rl_repo')

import numpy as np
import concourse.bass as bass
import concourse.mybir as mybir
from concourse.tile import TileContext
from concourse.vector_clock import ScopedClock

# ----------------------------------------------------------------------------
# Workarounds: the installed walrus accepts only 1 sync-wait per instruction.
# Split excess waits onto NoOp carriers (same engine, program order).
# ----------------------------------------------------------------------------
WAIT_LIMIT = 1


def _make_wait_nop(nc, engine, waits):
    nop = mybir.InstNoOp(name=nc.get_next_instruction_name(), ins=[], outs=[])
    nop.engine = engine
    nop.sync_info = mybir.SyncInfo(on_wait=list(waits), on_update=[])
    return nop


def _split_waits(nc, insts):
    out = []
    for inst in insts:
        si = inst.sync_info
        waits = list(si.on_wait) if (si is not None and si.on_wait) else []
        if len(waits) > WAIT_LIMIT and inst.engine != mybir.EngineType.Unassigned:
            excess, keep = waits[:-WAIT_LIMIT], waits[-WAIT_LIMIT:]
            si.on_wait = keep
            for i in range(0, len(excess), WAIT_LIMIT):
                out.append(_make_wait_nop(nc, inst.engine, excess[i:i + WAIT_LIMIT]))
        out.append(inst)
    return out


if not getattr(TileContext, "_wait_split_patched", False):
    _orig_lower = TileContext._lower_ordered_insts

    def _patched_lower(self, ordered):
        for bb in list(ordered.keys()):
            ordered[bb] = _split_waits(self.nc, ordered[bb])
        return _orig_lower(self, ordered)

    TileContext._lower_ordered_insts = _patched_lower

    def _patched_drain_and_barrier(self, tick_clock, wait_clock):
        drain_inst = self.nc.sync.drain()
        wait_clock.add_sem_waits(drain_inst.ins,
                                 ScopedClock({None: tick_clock.global_clock}))
        si = drain_inst.ins.sync_info
        waits = list(si.on_wait) if si and si.on_wait else []
        if len(waits) > WAIT_LIMIT:
            si.on_wait = waits[:WAIT_LIMIT]
            rest = waits[WAIT_LIMIT:]
            for i in range(0, len(rest), WAIT_LIMIT):
                extra = self.nc.sync.drain()
                esi = extra.ins.sync_info
                if esi is None:
                    extra.ins.sync_info = mybir.SyncInfo(
                        on_wait=rest[i:i + WAIT_LIMIT], on_update=[])
                else:
                    esi.on_wait = rest[i:i + WAIT_LIMIT]
        self.nc.all_engine_barrier()
        assert self.sems is not None
        popped = self.nc._tile_sem_poison_stack.pop()
        assert popped is self._sem_poison
        self.nc.clear_and_free_semaphores(list(self.sems.allocated().values()))
        self.nc.all_engine_barrier()

    TileContext._drain_and_barrier = _patched_drain_and_barrier
    TileContext._wait_split_patched = True


def _install_debug_hook():
    try:
        import libneuronxla
    except ImportError:
        return
    from concourse import bass2jax as _b2j
    _b2j.install_neuronx_cc_hook()
    _cur = libneuronxla.neuronx_cc
    if getattr(_cur, "_is_debug_hook", False):
        return

    def _debug_hook(*args, **kwargs):
        try:
            return _cur(*args, **kwargs)
        except BaseException:
            traceback.print_exc()
            with open('/tmp/hook_log.txt', 'a') as f:
                traceback.print_exc(file=f)
            raise

    _debug_hook._is_debug_hook = True
    libneuronxla.neuronx_cc = _debug_hook
    _b2j.neuronx_cc_hook = _debug_hook


_install_debug_hook()

# ----------------------------------------------------------------------------
# Problem constants
# ----------------------------------------------------------------------------
B, T_IN, T_OUT = 256, 168, 24
F, H, C = 64, 128, 8
PER = F // C          # 8 features per cluster
FPC = F // 8          # 8 features per core (== PER; core c owns cluster c)
G4 = 4 * H            # 512 gate rows
SCALE = 1.0 / np.sqrt(H)

f32 = mybir.dt.float32
f32r = mybir.dt.float32r
AF = mybir.ActivationFunctionType
ALU = mybir.AluOpType


# ----------------------------------------------------------------------------
# Kernel builder (parametric so small configs can be simulated quickly)
# ----------------------------------------------------------------------------
def build_kernel(t_in=T_IN, t_out=T_OUT, n_feat=FPC, n_cores=8, batch=B):
    assert batch % 2 == 0
    Bf = batch               # free-dim batch
    XCH = 21 if t_in % 21 == 0 else t_in   # x DMA chunk length (steps)
    n_ch = t_in // XCH

    nc = bass.Bass()

    # ---- DRAM I/O (per core). float32r tensors still take np.float32 arrays.
    xenc = nc.dram_tensor("xenc", [t_in, 9, Bf], f32r, kind="ExternalInput")
    e_l0w = nc.dram_tensor("e_l0w", [9, G4], f32r, kind="ExternalInput")
    e_l0h = nc.dram_tensor("e_l0h", [H, G4], f32r, kind="ExternalInput")
    e_l1h = nc.dram_tensor("e_l1h", [H, G4], f32r, kind="ExternalInput")
    e_l1g = nc.dram_tensor("e_l1g", [H, G4], f32r, kind="ExternalInput")
    e_l1b = nc.dram_tensor("e_l1b", [1, G4], f32r, kind="ExternalInput")

    d_ihb = nc.dram_tensor("d_ihb", [2, n_feat * G4], f32r, kind="ExternalInput")
    d_whh = nc.dram_tensor("d_whh", [H, n_feat * G4], f32r, kind="ExternalInput")
    d_fc1 = nc.dram_tensor("d_fc1", [H, n_feat * H], f32r, kind="ExternalInput")
    d_fc1b = nc.dram_tensor("d_fc1b", [H, n_feat], f32, kind="ExternalInput")
    d_M = nc.dram_tensor("d_M", [H, n_feat * H], f32r, kind="ExternalInput")
    d_w1 = nc.dram_tensor("d_w1", [H, 2 * n_feat], f32r, kind="ExternalInput")
    d_Wv = nc.dram_tensor("d_Wv", [H, n_feat * H], f32r, kind="ExternalInput")
    d_ow = nc.dram_tensor("d_ow", [H, n_feat * H], f32r, kind="ExternalInput")
    d_fc2 = nc.dram_tensor("d_fc2", [H, n_feat], f32r, kind="ExternalInput")
    d_beta = nc.dram_tensor("d_beta", [1, n_feat], f32, kind="ExternalInput")
    d_x0 = nc.dram_tensor("d_x0", [2, n_feat * Bf], f32r, kind="ExternalInput")
    d_const = nc.dram_tensor("d_const", [H, 2 * Bf], f32r, kind="ExternalInput")

    out_d = nc.dram_tensor("out", [t_out, n_feat, Bf], f32r, kind="ExternalOutput")

    with TileContext(nc) as tc:
        with tc.tile_pool(name="wgt", bufs=1) as wp, \
             tc.tile_pool(name="state", bufs=1) as sp, \
             tc.tile_pool(name="xe", bufs=2) as xp, \
             tc.tile_pool(name="work", bufs=3) as kp, \
             tc.tile_pool(name="gps", bufs=1, space="PSUM") as gps, \
             tc.tile_pool(name="mps", bufs=2, space="PSUM") as mps, \
             tc.tile_pool(name="acc", bufs=1, space="PSUM") as acc, \
             tc.tile_pool(name="dram", bufs=1, space="DRAM") as dp:

            # ---------------- constants / weights into SBUF ----------------
            w_l0w = wp.tile([9, G4], f32r, tag="w_l0w")
            w_l0h = wp.tile([H, G4], f32r, tag="w_l0h")
            w_l1h = wp.tile([H, G4], f32r, tag="w_l1h")
            w_l1g = wp.tile([H, G4], f32r, tag="w_l1g")
            w_l1b = wp.tile([1, G4], f32r, tag="w_l1b")
            nc.sync.dma_start(out=w_l0w[:], in_=e_l0w[:])
            nc.sync.dma_start(out=w_l0h[:], in_=e_l0h[:])
            nc.sync.dma_start(out=w_l1h[:], in_=e_l1h[:])
            nc.sync.dma_start(out=w_l1g[:], in_=e_l1g[:])
            nc.sync.dma_start(out=w_l1b[:], in_=e_l1b[:])

            w_ihb = wp.tile([2, n_feat * G4], f32r, tag="w_ihb")
            w_whh = wp.tile([H, n_feat * G4], f32r, tag="w_whh")
            w_fc1 = wp.tile([H, n_feat * H], f32r, tag="w_fc1")
            w_fc1b = wp.tile([H, n_feat], f32, tag="w_fc1b")
            w_M = wp.tile([H, n_feat * H], f32r, tag="w_M")
            w_w1 = wp.tile([H, 2 * n_feat], f32r, tag="w_w1")
            w_Wv = wp.tile([H, n_feat * H], f32r, tag="w_Wv")
            w_ow = wp.tile([H, n_feat * H], f32r, tag="w_ow")
            w_fc2 = wp.tile([H, n_feat], f32r, tag="w_fc2")
            w_beta = wp.tile([1, n_feat], f32, tag="w_beta")
            nc.sync.dma_start(out=w_ihb[:], in_=d_ihb[:])
            nc.sync.dma_start(out=w_whh[:], in_=d_whh[:])
            nc.sync.dma_start(out=w_fc1[:], in_=d_fc1[:])
            nc.sync.dma_start(out=w_fc1b[:], in_=d_fc1b[:])
            nc.sync.dma_start(out=w_M[:], in_=d_M[:])
            nc.sync.dma_start(out=w_w1[:], in_=d_w1[:])
            nc.sync.dma_start(out=w_Wv[:], in_=d_Wv[:])
            nc.sync.dma_start(out=w_ow[:], in_=d_ow[:])
            nc.sync.dma_start(out=w_fc2[:], in_=d_fc2[:])
            nc.sync.dma_start(out=w_beta[:], in_=d_beta[:])

            ones_row = wp.tile([1, Bf], f32r, tag="ones_row")
            ones_col = wp.tile([H, 1], f32r, tag="ones_col")
            nc.sync.dma_start(out=ones_row[:], in_=d_const[0:1, 0:Bf])
            nc.sync.dma_start(out=ones_col[:], in_=d_const[:, 0:1])

            # ---------------- states ----------------
            h0 = sp.tile([H, Bf], f32r, tag="h0")
            h1 = sp.tile([H, Bf], f32r, tag="h1")
            S0 = sp.tile([H, Bf], f32, tag="S0")
            S1 = sp.tile([H, Bf], f32, tag="S1")
            nc.sync.dma_start(out=h0[:], in_=d_const[:, Bf:2 * Bf])
            nc.sync.dma_start(out=h1[:], in_=d_const[:, Bf:2 * Bf])
            nc.vector.memset(S0[:], 0.0)
            nc.vector.memset(S1[:], 0.0)

            # ============================ ENCODER ============================
            def lstm_cell(g_ps, S, h, tag):
                """gate psum (H, 4B) laid [i|f|g|o] -> updates S (2c), h (2h)."""
                T = kp.tile([H, 4 * Bf], f32, tag="T")
                nc.scalar.activation(T[:], g_ps[:], AF.Tanh)
                m1 = kp.tile([H, Bf], f32, tag="m1")
                m2 = kp.tile([H, Bf], f32, tag="m2")
                # m1 = (T_f + 1) * S = 2 sig(f) * S ;  m2 = (T_i + 1) * T_g
                nc.vector.scalar_tensor_tensor(
                    m1[:], T[:, Bf:2 * Bf], 1.0, S[:], op0=ALU.add, op1=ALU.mult)
                nc.vector.scalar_tensor_tensor(
                    m2[:], T[:, 0:Bf], 1.0, T[:, 2 * Bf:3 * Bf],
                    op0=ALU.add, op1=ALU.mult)
                # S_new = 2c_new = 0.5*m1 + m2   (m1 = 2 sig(f) * (2c) )
                nc.vector.scalar_tensor_tensor(
                    S[:], m1[:], 0.5, m2[:], op0=ALU.mult, op1=ALU.add)
                th = kp.tile([H, Bf], f32, tag="th")
                nc.scalar.activation(th[:], S[:], AF.Tanh, scale=0.5)
                # h_new = 2h = (T_o + 1) * tanh(c)
                nc.vector.scalar_tensor_tensor(
                    h[:], T[:, 3 * Bf:4 * Bf], 1.0, th[:], op0=ALU.add, op1=ALU.mult)

            for ch in range(n_ch):
                xe = xp.tile([9, XCH * Bf], f32r, tag="xe")
                nc.sync.dma_start(
                    out=xe[:].rearrange("p (t b) -> p t b", b=Bf),
                    in_=xenc[ch * XCH:(ch + 1) * XCH, :, :].rearrange("t p b -> p t b"))
                for tl in range(XCH):
                    x_t = xe[:, tl * Bf:(tl + 1) * Bf]
                    # ---- layer 0
                    g_ps = gps.tile([H, 4 * Bf], f32, tag="g0")
                    for g in range(4):
                        o = g_ps[:, g * Bf:(g + 1) * Bf]
                        nc.tensor.matmul(o, w_l0w[:, g * H:(g + 1) * H], x_t,
                                         start=True, stop=False)
                        nc.tensor.matmul(o, w_l0h[:, g * H:(g + 1) * H], h0[:],
                                         start=False, stop=True)
                    lstm_cell(g_ps, S0, h0, "l0")
                    # ---- layer 1
                    g_ps1 = gps.tile([H, 4 * Bf], f32, tag="g1")
                    for g in range(4):
                        o = g_ps1[:, g * Bf:(g + 1) * Bf]
                        nc.tensor.matmul(o, w_l1h[:, g * H:(g + 1) * H], h0[:],
                                         start=True, stop=False)
                        nc.tensor.matmul(o, w_l1g[:, g * H:(g + 1) * H], h1[:],
                                         start=False, stop=False)
                        nc.tensor.matmul(o, w_l1b[:, g * H:(g + 1) * H], ones_row[:],
                                         start=False, stop=True)
                    lstm_cell(g_ps1, S1, h1, "l1")

            # ===================== ALLREDUCE (cluster mean) ==================
            cc_sb = sp.tile([H, 2 * Bf], f32, tag="cc_sb")
            cc_in = dp.tile([H, 2 * Bf], f32)
            cc_out = dp.tile([H, 2 * Bf], f32)
            nc.sync.dma_start(out=cc_in[:, 0:Bf], in_=h1[:].bitcast(f32))
            nc.sync.dma_start(out=cc_in[:, Bf:2 * Bf], in_=S1[:])
            nc.gpsimd.collective_compute(
                "AllReduce", ALU.add,
                replica_groups=[list(range(n_cores))],
                ins=[cc_in.opt()], outs=[cc_out.opt()])
            nc.sync.dma_start(out=cc_sb[:], in_=cc_out[:])

            # dec states (doubled): h' = 2*(hid+mean)/2 = h1/2 + hsum/16
            # (h1 is 2*hid; hsum is sum of 2*hid over 8 cores)
            dh = sp.tile([H, n_feat * Bf], f32r, tag="dh")
            dS = sp.tile([H, n_feat * Bf], f32, tag="dS")
            mh = kp.tile([H, Bf], f32, tag="mh")
            ms = kp.tile([H, Bf], f32, tag="ms")
            den = 2.0 * n_cores
            nc.vector.tensor_scalar_mul(mh[:], cc_sb[:, 0:Bf], 1.0 / den)
            nc.vector.tensor_scalar_mul(ms[:], cc_sb[:, Bf:2 * Bf], 1.0 / den)
            for f in range(n_feat):
                nc.vector.scalar_tensor_tensor(
                    dh[:, f * Bf:(f + 1) * Bf], h1[:].bitcast(f32), 0.5, mh[:],
                    op0=ALU.mult, op1=ALU.add)
                nc.vector.scalar_tensor_tensor(
                    dS[:, f * Bf:(f + 1) * Bf], S1[:], 0.5, ms[:],
                    op0=ALU.mult, op1=ALU.add)

            # ============================ DECODER ============================
            # feedback buffer at partition 0/1: row 0 = x per feature
            # (overwritten each step), row 1 = ones. Outputs are DMA'd to
            # DRAM once per step before the row is overwritten.
            xb = sp.tile([2, n_feat * Bf], f32r, tag="xb")
            nc.sync.dma_start(out=xb[:], in_=d_x0[:])

            for t in range(t_out):
                for f in range(n_feat):
                    hs = dh[:, f * Bf:(f + 1) * Bf]
                    Ss = dS[:, f * Bf:(f + 1) * Bf]
                    x_aug = xb[0:2, f * Bf:(f + 1) * Bf]
                    # ---- LSTM cell (input dim 1 + bias folded into K=2 mm)
                    g_ps = gps.tile([H, 4 * Bf], f32, tag="g0" if (t * n_feat + f) % 2 == 0 else "g1")
                    for g in range(4):
                        o = g_ps[:, g * Bf:(g + 1) * Bf]
                        nc.tensor.matmul(
                            o, w_ihb[:, f * G4 + g * H:f * G4 + (g + 1) * H],
                            x_aug, start=True, stop=False)
                        nc.tensor.matmul(
                            o, w_whh[:, f * G4 + g * H:f * G4 + (g + 1) * H],
                            hs, start=False, stop=True)
                    T = kp.tile([H, 4 * Bf], f32, tag="T")
                    nc.scalar.activation(T[:], g_ps[:], AF.Tanh)
                    m1 = kp.tile([H, Bf], f32, tag="m1")
                    m2 = kp.tile([H, Bf], f32, tag="m2")
                    nc.vector.scalar_tensor_tensor(
                        m1[:], T[:, Bf:2 * Bf], 1.0, Ss, op0=ALU.add, op1=ALU.mult)
                    nc.vector.scalar_tensor_tensor(
                        m2[:], T[:, 0:Bf], 1.0, T[:, 2 * Bf:3 * Bf],
                        op0=ALU.add, op1=ALU.mult)
                    nc.vector.scalar_tensor_tensor(
                        Ss, m1[:], 0.5, m2[:], op0=ALU.mult, op1=ALU.add)
                    th = kp.tile([H, Bf], f32, tag="th")
                    nc.scalar.activation(th[:], Ss, AF.Tanh, scale=0.5)
                    nc.vector.scalar_tensor_tensor(
                        hs, T[:, 3 * Bf:4 * Bf], 1.0, th[:], op0=ALU.add, op1=ALU.mult)

                    # ---- qkv = lrelu(fc1 @ h + b)   (fc1 pre-halved for 2h)
                    q_ps = mps.tile([H, Bf], f32, tag="m")
                    nc.tensor.matmul(q_ps[:], w_fc1[:, f * H:(f + 1) * H], hs,
                                     start=True, stop=True)
                    qkv = kp.tile([H, Bf], f32r, tag="qkv")
                    nc.scalar.activation(qkv[:], q_ps[:], AF.Lrelu,
                                         bias=w_fc1b[:, f:f + 1], alpha=0.01)

                    # ---- z = (Wk^T Wq) @ qkv ; per-key bias = w1 . qkv
                    z_ps = mps.tile([H, Bf], f32, tag="m")
                    nc.tensor.matmul(z_ps[:], w_M[:, f * H:(f + 1) * H], qkv[:],
                                     start=True, stop=True)
                    z = kp.tile([H, Bf], f32r, tag="z")
                    nc.vector.tensor_copy(z[:], z_ps[:])
                    sb_ps = mps.tile([H, 4], f32, tag="m")
                    for k in range(2):
                        nc.tensor.matmul(sb_ps[:, 2 * k:2 * k + 2],
                                         qkv[:, k * H:(k + 1) * H],
                                         w_w1[:, 2 * f:2 * f + 2], start=True, stop=True)
                    sbias = kp.tile([H, 4], f32, tag="sbias")
                    nc.vector.tensor_copy(sbias[:], sb_ps[:])

                    # ---- v = qkv^T @ Wv (per key-chunk), in (B,H) layout
                    v_ps = mps.tile([H, 2 * H], f32, tag="m")
                    for k in range(2):
                        nc.tensor.matmul(v_ps[:, k * H:(k + 1) * H],
                                         qkv[:, k * H:(k + 1) * H],
                                         w_Wv[:, f * H:(f + 1) * H],
                                         start=True, stop=True)
                    v = kp.tile([H, 2 * H], f32r, tag="v")
                    nc.vector.tensor_copy(v[:], v_ps[:])

                    # ---- scores_T (key-part, query-free) + exp
                    sc_ps = acc.tile([H, 2 * Bf], f32, tag="sc")
                    for k in range(2):
                        nc.tensor.matmul(sc_ps[:, k * Bf:(k + 1) * Bf],
                                         qkv[:, k * H:(k + 1) * H], z[:],
                                         start=True, stop=True)
                    expT = kp.tile([H, 2 * Bf], f32r, tag="expT")
                    for k in range(2):
                        nc.scalar.activation(expT[:, k * Bf:(k + 1) * Bf],
                                             sc_ps[:, k * Bf:(k + 1) * Bf],
                                             AF.Exp, bias=sbias[:, 2 * k:2 * k + 1], scale=SCALE)

                    # ---- colsum (1,B) and unnormalized ao_T = v^T @ expT
                    aocs = acc.tile([H, 2 * Bf], f32, tag="acc")
                    ao_ps = aocs[:, 0:Bf]
                    cs_ps = aocs[0:1, Bf:Bf + Bf]
                    for k in range(2):
                        nc.tensor.matmul(cs_ps, ones_col[:],
                                         expT[:, k * Bf:(k + 1) * Bf],
                                         start=(k == 0), stop=(k == 1))
                    for k in range(2):
                        nc.tensor.matmul(ao_ps, v[:, k * H:(k + 1) * H],
                                         expT[:, k * Bf:(k + 1) * Bf],
                                         start=(k == 0), stop=(k == 1))
                    recip = kp.tile([1, Bf], f32, tag="recip")
                    nc.vector.reciprocal(recip[:], cs_ps)
                    ao = kp.tile([H, Bf], f32r, tag="ao")
                    nc.vector.tensor_copy(ao[:], ao_ps)

                    # ---- out_w @ ao ; fc2 ; normalize ; lrelu(+beta)
                    a2_ps = mps.tile([H, Bf], f32, tag="m")
                    nc.tensor.matmul(a2_ps[:], w_ow[:, f * H:(f + 1) * H], ao[:],
                                     start=True, stop=True)
                    ao2 = kp.tile([H, Bf], f32r, tag="ao2")
                    nc.vector.tensor_copy(ao2[:], a2_ps[:])
                    y_ps = mps.tile([1, Bf], f32, tag="m")
                    nc.tensor.matmul(y_ps[:], w_fc2[:, f:f + 1], ao2[:],
                                     start=True, stop=True)
                    yn = kp.tile([1, Bf], f32, tag="yn")
                    nc.vector.tensor_tensor(yn[:], y_ps[:], recip[:], op=ALU.mult)
                    nc.scalar.activation(xb[0:1, f * Bf:(f + 1) * Bf],
                                         yn[:], AF.Lrelu, bias=w_beta[:, f:f + 1],
                                         alpha=0.01)

                # step outputs (== next x) to DRAM before they are overwritten
                nc.sync.dma_start(
                    out=out_d[t].rearrange("f b -> (f b)"),
                    in_=xb[0:1, :])

    return nc


# ----------------------------------------------------------------------------
# Host-side weight prep
# ----------------------------------------------------------------------------
def prep_inputs(inputs, t_in=T_IN, t_out=T_OUT, n_feat=FPC, n_cores=8, batch=B):
    """Build per-core in_maps from the full problem inputs."""
    x = np.asarray(inputs["x"], np.float32)
    in_maps = []
    # sigma trick scale for i,f,o rows (tanh(x/2)); g rows stay 1.0
    gate_scale = np.concatenate([
        np.full(H, 0.5, np.float32), np.full(H, 0.5, np.float32),
        np.ones(H, np.float32), np.full(H, 0.5, np.float32)])

    for c in range(n_cores):
        m = {}
        # ---------------- encoder (cluster c) ----------------
        xc = x[:batch, :t_in, c * PER:(c + 1) * PER]      # (B, T, 8)
        xe = np.empty((t_in, 9, batch), np.float32)
        xe[:, 0:8, :] = xc.transpose(1, 2, 0)
        xe[:, 8, :] = 1.0
        m["xenc"] = np.ascontiguousarray(xe)

        wih0 = np.asarray(inputs["enc_Wih0"][c], np.float32)   # (4H, PER)
        whh0 = np.asarray(inputs["enc_Whh0"][c], np.float32)   # (4H, H)
        b0 = np.asarray(inputs["enc_bih0"][c] + inputs["enc_bhh0"][c], np.float32)
        wih1 = np.asarray(inputs["enc_Wih1"][c], np.float32)
        whh1 = np.asarray(inputs["enc_Whh1"][c], np.float32)
        b1 = np.asarray(inputs["enc_bih1"][c] + inputs["enc_bhh1"][c], np.float32)

        gs = gate_scale[:, None]
        # L0: x-term lhsT rows = [Wih0^T ; bias], scaled by sigma trick
        l0w = np.zeros((9, G4), np.float32)
        l0w[0:8, :] = (wih0 * gs).T
        l0w[8, :] = b0 * gate_scale
        m["e_l0w"] = l0w
        # L0 h-term: h is doubled -> halve; plus sigma trick
        m["e_l0h"] = np.ascontiguousarray((whh0 * 0.5 * gs).T)
        # L1: input h0 doubled -> halve; sigma trick
        m["e_l1h"] = np.ascontiguousarray((wih1 * 0.5 * gs).T)
        m["e_l1g"] = np.ascontiguousarray((whh1 * 0.5 * gs).T)
        m["e_l1b"] = (b1 * gate_scale)[None, :]

        # ---------------- decoder (features c*8 .. c*8+n_feat) --------------
        ihb = np.zeros((2, n_feat * G4), np.float32)
        whh = np.zeros((H, n_feat * G4), np.float32)
        fc1 = np.zeros((H, n_feat * H), np.float32)
        fc1b = np.zeros((H, n_feat), np.float32)
        Mt = np.zeros((H, n_feat * H), np.float32)
        w1 = np.zeros((H, 2 * n_feat), np.float32)
        Wv = np.zeros((H, n_feat * H), np.float32)
        ow = np.zeros((H, n_feat * H), np.float32)
        fc2 = np.zeros((H, n_feat), np.float32)
        beta = np.zeros((1, n_feat), np.float32)
        x0 = np.ones((2, n_feat * batch), np.float32)

        for j in range(n_feat):
            fi = c * PER + j
            dwih = np.asarray(inputs["dec_Wih"][fi], np.float32)   # (4H, 1)
            dwhh = np.asarray(inputs["dec_Whh"][fi], np.float32)   # (4H, H)
            db = np.asarray(inputs["dec_bih"][fi] + inputs["dec_bhh"][fi], np.float32)
            aw = np.asarray(inputs["attn_in_w"][fi], np.float32)   # (3H, H)
            ab = np.asarray(inputs["attn_in_b"][fi], np.float32)   # (3H,)
            aow = np.asarray(inputs["attn_out_w"][fi], np.float32)  # (H, H)
            aob = np.asarray(inputs["attn_out_b"][fi], np.float32)  # (H,)
            f1w = np.asarray(inputs["fc1_w"][fi], np.float32)      # (H, H)
            f1b = np.asarray(inputs["fc1_b"][fi], np.float32)      # (H,)
            f2w = np.asarray(inputs["fc2_w"][fi], np.float32)      # (1, H)
            f2b = np.asarray(inputs["fc2_b"][fi], np.float32)      # (1,)

            Wq, Wk, Wvv = aw[0:H], aw[H:2 * H], aw[2 * H:3 * H]
            bq, bk, bv = ab[0:H], ab[H:2 * H], ab[2 * H:3 * H]

            ihb[0, j * G4:(j + 1) * G4] = dwih[:, 0] * gate_scale
            ihb[1, j * G4:(j + 1) * G4] = db * gate_scale
            whh[:, j * G4:(j + 1) * G4] = (dwhh * 0.5 * gs).T
            fc1[:, j * H:(j + 1) * H] = (f1w * 0.5).T
            fc1b[:, j] = f1b
            Mt[:, j * H:(j + 1) * H] = (Wk.T @ Wq).T
            w1[:, 2 * j] = SCALE * (Wk.T @ bq)
            w1[:, 2 * j + 1] = w1[:, 2 * j]
            Wv[:, j * H:(j + 1) * H] = Wvv.T
            ow[:, j * H:(j + 1) * H] = aow.T
            fc2[:, j] = f2w[0]
            beta[0, j] = float(f2w[0] @ (aow @ bv + aob) + f2b[0])
            x0[0, j * batch:(j + 1) * batch] = x[:batch, -1, fi]

        const = np.zeros((H, 2 * batch), np.float32)
        const[:, 0:batch] = 1.0
        m.update(d_ihb=ihb, d_whh=whh, d_fc1=fc1, d_fc1b=fc1b, d_M=Mt,
                 d_w1=w1, d_Wv=Wv, d_ow=ow, d_fc2=fc2, d_beta=beta, d_x0=x0,
                 d_const=const)
        in_maps.append(m)
    return in_maps


def assemble_output(results, t_out=T_OUT, n_feat=FPC, batch=B):
    out = np.empty((batch, t_out, len(results) * n_feat), np.float32)
    for c, r in enumerate(results):
        # r["out"]: (t_out, n_feat, B)
        out[:, :, c * n_feat:(c + 1) * n_feat] = r["out"].transpose(2, 0, 1)
    return out


_cached = {}


def kernel(**inputs) -> np.ndarray:
    from concourse.bass_utils import run_bass_kernel_spmd
    key = "full"
    if key not in _cached:
        _cached[key] = build_kernel()
    nc = _cached[key]
    in_maps = prep_inputs(inputs)
    res = run_bass_kernel_spmd(nc, in_maps, core_ids=list(range(8)))
    return assemble_output(res.results)


# revision 25
# speedup vs baseline: 1.0317x; 1.0317x over previous
"""Trainium2 Bass kernel for nn_ClusteringMultiTaskLSTM.

Self-contained: builds + runs an SPMD kernel on 8 NeuronCores.
  - Encoder: one cluster per core (C=8), 2-layer LSTM over T_IN steps,
    transposed layout (dim, B), fp32r (TF32) matmuls.
  - AllReduce of final encoder states across cores (cluster mean).
  - Decoder: 8 per-feature decoders per core (features 8c..8c+7 all belong
    to cluster c), T_OUT steps of LSTM cell + batch-axis attention + fc2.

Math tricks (exact, not approximations):
  - sigmoid(x) = (1 + tanh(x/2))/2 -> all gates use one tanh table set; the
    1/2 input scale is folded into i,f,o weight rows host-side; the output
    affine is folded into fused scalar_tensor_tensor ops via a doubled
    state: S = 2c, hS = 2h, with h-consuming weights pre-halved host-side.
  - attention scores: q.k = qkv^T (Wk^T Wq) qkv + per-key term + per-query
    terms; per-query terms are softmax-invariant and dropped; the per-key
    term becomes the exp() per-partition bias; softmax normalization is
    deferred through the (linear) attn->out_w->fc2 chain and applied as a
    single multiply by 1/colsum on the (1,B) fc2 output.
  - v bias + out_w bias + fc2 bias collapse to one scalar beta per feature.
"""
import sys, traceback
if '/opt/trn_rl_repo' not in sys.path:
    sys.path.insert(0, '/opt/trn_rl_repo')

import numpy as np
import concourse.bass as bass
import concourse.mybir as mybir
from concourse.tile import TileContext
from concourse.vector_clock import ScopedClock

# ----------------------------------------------------------------------------
# Workarounds: the installed walrus accepts only 1 sync-wait per instruction.
# Split excess waits onto NoOp carriers (same engine, program order).
# ----------------------------------------------------------------------------
WAIT_LIMIT = 1


def _make_wait_nop(nc, engine, waits):
    nop = mybir.InstNoOp(name=nc.get_next_instruction_name(), ins=[], outs=[])
    nop.engine = engine
    nop.sync_info = mybir.SyncInfo(on_wait=list(waits), on_update=[])
    return nop


def _split_waits(nc, insts):
    out = []
    for inst in insts:
        si = inst.sync_info
        waits = list(si.on_wait) if (si is not None and si.on_wait) else []
        if len(waits) > WAIT_LIMIT and inst.engine != mybir.EngineType.Unassigned:
            excess, keep = waits[:-WAIT_LIMIT], waits[-WAIT_LIMIT:]
            si.on_wait = keep
            for i in range(0, len(excess), WAIT_LIMIT):
                out.append(_make_wait_nop(nc, inst.engine, excess[i:i + WAIT_LIMIT]))
        out.append(inst)
    return out


if not getattr(TileContext, "_wait_split_patched", False):
    _orig_lower = TileContext._lower_ordered_insts

    def _patched_lower(self, ordered):
        for bb in list(ordered.keys()):
            ordered[bb] = _split_waits(self.nc, ordered[bb])
        return _orig_lower(self, ordered)

    TileContext._lower_ordered_insts = _patched_lower

    def _patched_drain_and_barrier(self, tick_clock, wait_clock):
        drain_inst = self.nc.sync.drain()
        wait_clock.add_sem_waits(drain_inst.ins,
                                 ScopedClock({None: tick_clock.global_clock}))
        si = drain_inst.ins.sync_info
        waits = list(si.on_wait) if si and si.on_wait else []
        if len(waits) > WAIT_LIMIT:
            si.on_wait = waits[:WAIT_LIMIT]
            rest = waits[WAIT_LIMIT:]
            for i in range(0, len(rest), WAIT_LIMIT):
                extra = self.nc.sync.drain()
                esi = extra.ins.sync_info
                if esi is None:
                    extra.ins.sync_info = mybir.SyncInfo(
                        on_wait=rest[i:i + WAIT_LIMIT], on_update=[])
                else:
                    esi.on_wait = rest[i:i + WAIT_LIMIT]
        self.nc.all_engine_barrier()
        assert self.sems is not None
        popped = self.nc._tile_sem_poison_stack.pop()
        assert popped is self._sem_poison
        self.nc.clear_and_free_semaphores(list(self.sems.allocated().values()))
        self.nc.all_engine_barrier()

    TileContext._drain_and_barrier = _patched_drain_and_barrier
    TileContext._wait_split_patched = True


def _install_debug_hook():
    try:
        import libneuronxla
    except ImportError:
        return
    from concourse import bass2jax as _b2j
    _b2j.install_neuronx_cc_hook()
    _cur = libneuronxla.neuronx_cc
    if getattr(_cur, "_is_debug_hook", False):
        return

    def _debug_hook(*args, **kwargs):
        try:
            return _cur(*args, **kwargs)
        except BaseException:
            traceback.print_exc()
            with open('/tmp/hook_log.txt', 'a') as f:
                traceback.print_exc(file=f)
            raise

    _debug_hook._is_debug_hook = True
    libneuronxla.neuronx_cc = _debug_hook
    _b2j.neuronx_cc_hook = _debug_hook


_install_debug_hook()

# ----------------------------------------------------------------------------
# Problem constants
# ----------------------------------------------------------------------------
B, T_IN, T_OUT = 256, 168, 24
F, H, C = 64, 128, 8
PER = F // C          # 8 features per cluster
FPC = F // 8          # 8 features per core (== PER; core c owns cluster c)
G4 = 4 * H            # 512 gate rows
SCALE = 1.0 / np.sqrt(H)

f32 = mybir.dt.float32
f32r = mybir.dt.float32r
AF = mybir.ActivationFunctionType
ALU = mybir.AluOpType


# ----------------------------------------------------------------------------
# Kernel builder (parametric so small configs can be simulated quickly)
# ----------------------------------------------------------------------------
def build_kernel(t_in=T_IN, t_out=T_OUT, n_feat=FPC, n_cores=8, batch=B):
    assert batch % 2 == 0
    Bf = batch               # free-dim batch
    XCH = 21 if t_in % 21 == 0 else t_in   # x DMA chunk length (steps)
    n_ch = t_in // XCH

    nc = bass.Bass()

    # ---- DRAM I/O (per core). float32r tensors still take np.float32 arrays.
    xenc = nc.dram_tensor("xenc", [t_in, 9, Bf], f32r, kind="ExternalInput")
    e_l0w = nc.dram_tensor("e_l0w", [9, G4], f32r, kind="ExternalInput")
    e_l0h = nc.dram_tensor("e_l0h", [H, G4], f32r, kind="ExternalInput")
    e_l1h = nc.dram_tensor("e_l1h", [H, G4], f32r, kind="ExternalInput")
    e_l1g = nc.dram_tensor("e_l1g", [H, G4], f32r, kind="ExternalInput")
    e_l1b = nc.dram_tensor("e_l1b", [1, G4], f32r, kind="ExternalInput")

    d_ihb = nc.dram_tensor("d_ihb", [2, n_feat * G4], f32r, kind="ExternalInput")
    d_whh = nc.dram_tensor("d_whh", [H, n_feat * G4], f32r, kind="ExternalInput")
    d_fc1 = nc.dram_tensor("d_fc1", [H, n_feat * H], f32r, kind="ExternalInput")
    d_fc1b = nc.dram_tensor("d_fc1b", [H, n_feat], f32, kind="ExternalInput")
    d_M = nc.dram_tensor("d_M", [H, n_feat * H], f32r, kind="ExternalInput")
    d_w1 = nc.dram_tensor("d_w1", [H, 2 * n_feat], f32r, kind="ExternalInput")
    d_Wv = nc.dram_tensor("d_Wv", [H, n_feat * H], f32r, kind="ExternalInput")
    d_ow = nc.dram_tensor("d_ow", [H, n_feat * H], f32r, kind="ExternalInput")
    d_fc2 = nc.dram_tensor("d_fc2", [H, n_feat], f32r, kind="ExternalInput")
    d_beta = nc.dram_tensor("d_beta", [1, n_feat], f32, kind="ExternalInput")
    d_x0 = nc.dram_tensor("d_x0", [2, n_feat * Bf], f32r, kind="ExternalInput")
    d_const = nc.dram_tensor("d_const", [H, 2 * Bf], f32r, kind="ExternalInput")

    out_d = nc.dram_tensor("out", [t_out, n_feat, Bf], f32r, kind="ExternalOutput")

    with TileContext(nc) as tc:
        with tc.tile_pool(name="wgt", bufs=1) as wp, \
             tc.tile_pool(name="state", bufs=1) as sp, \
             tc.tile_pool(name="xe", bufs=2) as xp, \
             tc.tile_pool(name="work", bufs=3) as kp, \
             tc.tile_pool(name="gps", bufs=1, space="PSUM") as gps, \
             tc.tile_pool(name="mps", bufs=2, space="PSUM") as mps, \
             tc.tile_pool(name="acc", bufs=1, space="PSUM") as acc, \
             tc.tile_pool(name="dram", bufs=1, space="DRAM") as dp:

            # ---------------- constants / weights into SBUF ----------------
            w_l0w = wp.tile([9, G4], f32r, tag="w_l0w")
            w_l0h = wp.tile([H, G4], f32r, tag="w_l0h")
            w_l1h = wp.tile([H, G4], f32r, tag="w_l1h")
            w_l1g = wp.tile([H, G4], f32r, tag="w_l1g")
            w_l1b = wp.tile([1, G4], f32r, tag="w_l1b")
            nc.sync.dma_start(out=w_l0w[:], in_=e_l0w[:])
            nc.sync.dma_start(out=w_l0h[:], in_=e_l0h[:])
            nc.sync.dma_start(out=w_l1h[:], in_=e_l1h[:])
            nc.sync.dma_start(out=w_l1g[:], in_=e_l1g[:])
            nc.sync.dma_start(out=w_l1b[:], in_=e_l1b[:])

            w_ihb = wp.tile([2, n_feat * G4], f32r, tag="w_ihb")
            w_whh = wp.tile([H, n_feat * G4], f32r, tag="w_whh")
            w_fc1 = wp.tile([H, n_feat * H], f32r, tag="w_fc1")
            w_fc1b = wp.tile([H, n_feat], f32, tag="w_fc1b")
            w_M = wp.tile([H, n_feat * H], f32r, tag="w_M")
            w_w1 = wp.tile([H, 2 * n_feat], f32r, tag="w_w1")
            w_Wv = wp.tile([H, n_feat * H], f32r, tag="w_Wv")
            w_ow = wp.tile([H, n_feat * H], f32r, tag="w_ow")
            w_fc2 = wp.tile([H, n_feat], f32r, tag="w_fc2")
            w_beta = wp.tile([1, n_feat], f32, tag="w_beta")
            nc.sync.dma_start(out=w_ihb[:], in_=d_ihb[:])
            nc.sync.dma_start(out=w_whh[:], in_=d_whh[:])
            nc.sync.dma_start(out=w_fc1[:], in_=d_fc1[:])
            nc.sync.dma_start(out=w_fc1b[:], in_=d_fc1b[:])
            nc.sync.dma_start(out=w_M[:], in_=d_M[:])
            nc.sync.dma_start(out=w_w1[:], in_=d_w1[:])
            nc.sync.dma_start(out=w_Wv[:], in_=d_Wv[:])
            nc.sync.dma_start(out=w_ow[:], in_=d_ow[:])
            nc.sync.dma_start(out=w_fc2[:], in_=d_fc2[:])
            nc.sync.dma_start(out=w_beta[:], in_=d_beta[:])

            ones_row = wp.tile([1, Bf], f32r, tag="ones_row")
            ones_col = wp.tile([H, 1], f32r, tag="ones_col")
            nc.sync.dma_start(out=ones_row[:], in_=d_const[0:1, 0:Bf])
            nc.sync.dma_start(out=ones_col[:], in_=d_const[:, 0:1])

            # ---------------- states ----------------
            h0 = sp.tile([H, Bf], f32r, tag="h0")
            h1 = sp.tile([H, Bf], f32r, tag="h1")
            S0 = sp.tile([H, Bf], f32, tag="S0")
            S1 = sp.tile([H, Bf], f32, tag="S1")
            nc.sync.dma_start(out=h0[:], in_=d_const[:, Bf:2 * Bf])
            nc.sync.dma_start(out=h1[:], in_=d_const[:, Bf:2 * Bf])
            nc.vector.memset(S0[:], 0.0)
            nc.vector.memset(S1[:], 0.0)

            # ============================ ENCODER ============================
            # Generator-based emission: ops of independent cells are
            # interleaved so each engine's in-order stream has independent
            # work to fill dependency gaps (software pipelining).
            uid = [0]

            def tl_(shape, dt_, tag):
                uid[0] += 1
                return kp.tile(shape, dt_, tag=tag, name=f"{tag}_{uid[0]}")

            def drive(gens, window=2):
                from collections import deque
                q = deque(gens)
                active = []
                while q or active:
                    while q and len(active) < window:
                        active.append(q.popleft())
                    for g in list(active):
                        try:
                            next(g)
                        except StopIteration:
                            active.remove(g)

            def lstm_cell_gen(g_tag, emit_mms, S, h):
                uid[0] += 1
                g_ps = gps.tile([H, 4 * Bf], f32, tag=g_tag,
                                name=f"g_{g_tag}_{uid[0]}")
                for _ in emit_mms(g_ps):
                    yield
                T = tl_([H, 4 * Bf], f32, "T")
                nc.scalar.activation(T[:], g_ps[:], AF.Tanh)
                yield
                m1 = tl_([H, Bf], f32, "m1")
                m2 = tl_([H, Bf], f32, "m2")
                nc.vector.scalar_tensor_tensor(
                    m1[:], T[:, Bf:2 * Bf], 1.0, S[:], op0=ALU.add, op1=ALU.mult)
                yield
                nc.vector.scalar_tensor_tensor(
                    m2[:], T[:, 0:Bf], 1.0, T[:, 2 * Bf:3 * Bf],
                    op0=ALU.add, op1=ALU.mult)
                yield
                nc.vector.scalar_tensor_tensor(
                    S[:], m1[:], 0.5, m2[:], op0=ALU.mult, op1=ALU.add)
                yield
                th = tl_([H, Bf], f32, "th")
                nc.scalar.activation(th[:], S[:], AF.Tanh, scale=0.5)
                yield
                nc.vector.scalar_tensor_tensor(
                    h[:], T[:, 3 * Bf:4 * Bf], 1.0, th[:], op0=ALU.add, op1=ALU.mult)
                yield

            xe_tiles = {}

            def get_xt(t):
                ch, tl = t // XCH, t % XCH
                if tl == 0:
                    uid[0] += 1
                    xe = xp.tile([9, XCH * Bf], f32r, tag="xe",
                                 name=f"xe_{uid[0]}")
                    xe_tiles[ch] = xe
                    nc.sync.dma_start(
                        out=xe[:].rearrange("p (t b) -> p t b", b=Bf),
                        in_=xenc[ch * XCH:(ch + 1) * XCH, :, :].rearrange(
                            "t p b -> p t b"))
                return xe_tiles[ch][:, tl * Bf:(tl + 1) * Bf]

            def cell_chain(g_ps, S, h):
                """ACT/DVE tail of one LSTM cell as a generator."""
                T = tl_([H, 4 * Bf], f32, "T")
                nc.scalar.activation(T[:], g_ps[:], AF.Tanh)
                yield
                m1 = tl_([H, Bf], f32, "m1")
                m2 = tl_([H, Bf], f32, "m2")
                nc.vector.scalar_tensor_tensor(
                    m1[:], T[:, Bf:2 * Bf], 1.0, S[:], op0=ALU.add, op1=ALU.mult)
                yield
                nc.vector.scalar_tensor_tensor(
                    m2[:], T[:, 0:Bf], 1.0, T[:, 2 * Bf:3 * Bf],
                    op0=ALU.add, op1=ALU.mult)
                yield
                nc.vector.scalar_tensor_tensor(
                    S[:], m1[:], 0.5, m2[:], op0=ALU.mult, op1=ALU.add)
                yield
                th = tl_([H, Bf], f32, "th")
                nc.scalar.activation(th[:], S[:], AF.Tanh, scale=0.5)
                yield
                nc.vector.scalar_tensor_tensor(
                    h[:], T[:, 3 * Bf:4 * Bf], 1.0, th[:], op0=ALU.add, op1=ALU.mult)
                yield

            # software-pipelined pair: B = L0(t+1) [short path to next pair],
            # A = L1(t). Independent mms first; h0-dependent mms last.
            def enc_pair(t):
                uid[0] += 1
                gA = gps.tile([H, 4 * Bf], f32, tag="g1", name=f"gA_{uid[0]}")
                gB = None
                if t + 1 < t_in:
                    x_t = get_xt(t + 1)
                    uid[0] += 1
                    gB = gps.tile([H, 4 * Bf], f32, tag="g0", name=f"gB_{uid[0]}")
                # phase A: independent matmuls (one accumulation group per
                # tile: start only on the tile's first mm, stop on its last)
                for g in range(4):
                    o = gA[:, g * Bf:(g + 1) * Bf]
                    nc.tensor.matmul(o, w_l1g[:, g * H:(g + 1) * H], h1[:],
                                     start=(g % 2 == 0), stop=False)
                    nc.tensor.matmul(o, w_l1b[:, g * H:(g + 1) * H],
                                     ones_row[:], start=False, stop=False)
                    if gB is not None:
                        nc.tensor.matmul(gB[:, g * Bf:(g + 1) * Bf],
                                         w_l0w[:, g * H:(g + 1) * H], x_t,
                                         start=(g % 2 == 0), stop=False)
                # phase B: h0-dependent matmuls (L0 first: next pair's input)
                if gB is not None:
                    for g in range(4):
                        nc.tensor.matmul(gB[:, g * Bf:(g + 1) * Bf],
                                         w_l0h[:, g * H:(g + 1) * H], h0[:],
                                         start=False, stop=(g % 2 == 1))
                for g in range(4):
                    nc.tensor.matmul(gA[:, g * Bf:(g + 1) * Bf],
                                     w_l1h[:, g * H:(g + 1) * H], h0[:],
                                     start=False, stop=(g % 2 == 1))
                # chains interleaved, L0' ops leading
                chains = [cell_chain(gA, S1, h1)]
                if gB is not None:
                    chains.insert(0, cell_chain(gB, S0, h0))
                drive(chains)

            # prologue: L0(0) alone
            x0t = get_xt(0)
            uid[0] += 1
            g00 = gps.tile([H, 4 * Bf], f32, tag="g0", name=f"g00_{uid[0]}")
            for g in range(4):
                o = g00[:, g * Bf:(g + 1) * Bf]
                nc.tensor.matmul(o, w_l0w[:, g * H:(g + 1) * H], x0t,
                                 start=True, stop=False)
                nc.tensor.matmul(o, w_l0h[:, g * H:(g + 1) * H], h0[:],
                                 start=False, stop=True)
            drive([cell_chain(g00, S0, h0)])
            for t in range(t_in):
                enc_pair(t)

            # ===================== ALLREDUCE (cluster mean) ==================
            cc_sb = sp.tile([H, 2 * Bf], f32, tag="cc_sb")
            cc_in = dp.tile([H, 2 * Bf], f32)
            cc_out = dp.tile([H, 2 * Bf], f32)
            nc.sync.dma_start(out=cc_in[:, 0:Bf], in_=h1[:].bitcast(f32))
            nc.sync.dma_start(out=cc_in[:, Bf:2 * Bf], in_=S1[:])
            nc.gpsimd.collective_compute(
                "AllReduce", ALU.add,
                replica_groups=[list(range(n_cores))],
                ins=[cc_in.opt()], outs=[cc_out.opt()])
            nc.sync.dma_start(out=cc_sb[:], in_=cc_out[:])

            # dec states (doubled): h' = 2*(hid+mean)/2 = h1/2 + hsum/16
            # (h1 is 2*hid; hsum is sum of 2*hid over 8 cores)
            dh = sp.tile([H, n_feat * Bf], f32r, tag="dh")
            dS = sp.tile([H, n_feat * Bf], f32, tag="dS")
            mh = kp.tile([H, Bf], f32, tag="mh")
            ms = kp.tile([H, Bf], f32, tag="ms")
            den = 2.0 * n_cores
            nc.vector.tensor_scalar_mul(mh[:], cc_sb[:, 0:Bf], 1.0 / den)
            nc.vector.tensor_scalar_mul(ms[:], cc_sb[:, Bf:2 * Bf], 1.0 / den)
            for f in range(n_feat):
                nc.vector.scalar_tensor_tensor(
                    dh[:, f * Bf:(f + 1) * Bf], h1[:].bitcast(f32), 0.5, mh[:],
                    op0=ALU.mult, op1=ALU.add)
                nc.vector.scalar_tensor_tensor(
                    dS[:, f * Bf:(f + 1) * Bf], S1[:], 0.5, ms[:],
                    op0=ALU.mult, op1=ALU.add)

            # ============================ DECODER ============================
            # feedback buffer at partition 0/1: row 0 = x per feature
            # (overwritten each step), row 1 = ones. Outputs are DMA'd to
            # DRAM once per step before the row is overwritten.
            xb = sp.tile([2, n_feat * Bf], f32r, tag="xb")
            nc.sync.dma_start(out=xb[:], in_=d_x0[:])

            def mtile(shape, tag="m"):
                uid[0] += 1
                return mps.tile(shape, f32, tag=tag, name=f"{tag}_{uid[0]}")

            def featstep(t, f):
                hs = dh[:, f * Bf:(f + 1) * Bf]
                Ss = dS[:, f * Bf:(f + 1) * Bf]
                x_aug = xb[0:2, f * Bf:(f + 1) * Bf]
                # ---- LSTM cell (input dim 1 + bias folded into K=2 mm)
                uid[0] += 1
                g_tag = "g0" if (t * n_feat + f) % 2 == 0 else "g1"
                g_ps = gps.tile([H, 4 * Bf], f32, tag=g_tag,
                                name=f"dg_{uid[0]}")
                for g in range(4):
                    o = g_ps[:, g * Bf:(g + 1) * Bf]
                    nc.tensor.matmul(
                        o, w_ihb[:, f * G4 + g * H:f * G4 + (g + 1) * H],
                        x_aug, start=True, stop=False)
                    nc.tensor.matmul(
                        o, w_whh[:, f * G4 + g * H:f * G4 + (g + 1) * H],
                        hs, start=False, stop=True)
                    yield
                T = tl_([H, 4 * Bf], f32, "T")
                nc.scalar.activation(T[:], g_ps[:], AF.Tanh)
                yield
                m1 = tl_([H, Bf], f32, "m1")
                m2 = tl_([H, Bf], f32, "m2")
                nc.vector.scalar_tensor_tensor(
                    m1[:], T[:, Bf:2 * Bf], 1.0, Ss, op0=ALU.add, op1=ALU.mult)
                yield
                nc.vector.scalar_tensor_tensor(
                    m2[:], T[:, 0:Bf], 1.0, T[:, 2 * Bf:3 * Bf],
                    op0=ALU.add, op1=ALU.mult)
                yield
                nc.vector.scalar_tensor_tensor(
                    Ss, m1[:], 0.5, m2[:], op0=ALU.mult, op1=ALU.add)
                yield
                th = tl_([H, Bf], f32, "th")
                nc.scalar.activation(th[:], Ss, AF.Tanh, scale=0.5)
                yield
                nc.vector.scalar_tensor_tensor(
                    hs, T[:, 3 * Bf:4 * Bf], 1.0, th[:], op0=ALU.add, op1=ALU.mult)
                yield

                # ---- qkv = lrelu(fc1 @ h + b)   (fc1 pre-halved for 2h)
                q_ps = mtile([H, Bf])
                nc.tensor.matmul(q_ps[:], w_fc1[:, f * H:(f + 1) * H], hs,
                                 start=True, stop=True)
                yield
                qkv = tl_([H, Bf], f32r, "qkv")
                nc.scalar.activation(qkv[:], q_ps[:], AF.Lrelu,
                                     bias=w_fc1b[:, f:f + 1], alpha=0.01)
                yield

                # ---- z = (Wk^T Wq) @ qkv ; per-key bias = w1 . qkv
                z_ps = mtile([H, Bf])
                nc.tensor.matmul(z_ps[:], w_M[:, f * H:(f + 1) * H], qkv[:],
                                 start=True, stop=True)
                yield
                z = tl_([H, Bf], f32r, "z")
                nc.vector.tensor_copy(z[:], z_ps[:])
                yield
                sb_ps = mtile([H, 4])
                for k in range(2):
                    nc.tensor.matmul(sb_ps[:, 2 * k:2 * k + 2],
                                     qkv[:, k * H:(k + 1) * H],
                                     w_w1[:, 2 * f:2 * f + 2], start=True, stop=True)
                yield
                sbias = tl_([H, 4], f32, "sbias")
                nc.vector.tensor_copy(sbias[:], sb_ps[:])
                yield

                # ---- v = qkv^T @ Wv (per key-chunk), in (B,H) layout
                v_ps = mtile([H, 2 * H])
                for k in range(2):
                    nc.tensor.matmul(v_ps[:, k * H:(k + 1) * H],
                                     qkv[:, k * H:(k + 1) * H],
                                     w_Wv[:, f * H:(f + 1) * H],
                                     start=True, stop=True)
                    yield
                v = tl_([H, 2 * H], f32r, "v")
                nc.vector.tensor_copy(v[:], v_ps[:])
                yield

                # ---- scores_T (key-part, query-free) + exp
                uid[0] += 1
                sc_ps = acc.tile([H, 2 * Bf], f32, tag="sc", name=f"sc_{uid[0]}")
                for k in range(2):
                    nc.tensor.matmul(sc_ps[:, k * Bf:(k + 1) * Bf],
                                     qkv[:, k * H:(k + 1) * H], z[:],
                                     start=True, stop=True)
                    yield
                expT = tl_([H, 2 * Bf], f32r, "expT")
                for k in range(2):
                    nc.scalar.activation(expT[:, k * Bf:(k + 1) * Bf],
                                         sc_ps[:, k * Bf:(k + 1) * Bf],
                                         AF.Exp, bias=sbias[:, 2 * k:2 * k + 1],
                                         scale=SCALE)
                    yield

                # ---- colsum (1,B) and unnormalized ao_T = v^T @ expT
                uid[0] += 1
                aocs = acc.tile([H, 2 * Bf], f32, tag="acc", name=f"acc_{uid[0]}")
                ao_ps = aocs[:, 0:Bf]
                cs_ps = aocs[0:1, Bf:Bf + Bf]
                for k in range(2):
                    nc.tensor.matmul(cs_ps, ones_col[:],
                                     expT[:, k * Bf:(k + 1) * Bf],
                                     start=(k == 0), stop=(k == 1))
                yield
                for k in range(2):
                    nc.tensor.matmul(ao_ps, v[:, k * H:(k + 1) * H],
                                     expT[:, k * Bf:(k + 1) * Bf],
                                     start=(k == 0), stop=(k == 1))
                    yield
                recip = tl_([1, Bf], f32, "recip")
                nc.vector.reciprocal(recip[:], cs_ps)
                yield
                ao = tl_([H, Bf], f32r, "ao")
                nc.vector.tensor_copy(ao[:], ao_ps)
                yield

                # ---- out_w @ ao ; fc2 ; normalize ; lrelu(+beta)
                a2_ps = mtile([H, Bf])
                nc.tensor.matmul(a2_ps[:], w_ow[:, f * H:(f + 1) * H], ao[:],
                                 start=True, stop=True)
                yield
                ao2 = tl_([H, Bf], f32r, "ao2")
                nc.vector.tensor_copy(ao2[:], a2_ps[:])
                yield
                y_ps = mtile([1, Bf])
                nc.tensor.matmul(y_ps[:], w_fc2[:, f:f + 1], ao2[:],
                                 start=True, stop=True)
                yield
                yn = tl_([1, Bf], f32, "yn")
                nc.vector.tensor_tensor(yn[:], y_ps[:], recip[:], op=ALU.mult)
                yield
                nc.scalar.activation(xb[0:1, f * Bf:(f + 1) * Bf],
                                     yn[:], AF.Lrelu, bias=w_beta[:, f:f + 1],
                                     alpha=0.01)
                yield

            done_count = {}

            def featstep_wrapped(t, f):
                for _ in featstep(t, f):
                    yield
                # last-completing featstep of step t emits the step-output
                # DMA: all step-t x-writes are emitted (deps captured), no
                # step-t+1 x-write is emitted yet (WAR ordered after).
                done_count[t] = done_count.get(t, 0) + 1
                if done_count[t] == n_feat:
                    nc.sync.dma_start(
                        out=out_d[t].rearrange("f b -> (f b)"),
                        in_=xb[0:1, :])

            dec_stream = []
            for t in range(t_out):
                for f in range(n_feat):
                    dec_stream.append(featstep_wrapped(t, f))
            import os
            drive(dec_stream, window=min(int(os.environ.get('DEC_WIN', '2')), n_feat))

    return nc


# ----------------------------------------------------------------------------
# Host-side weight prep
# ----------------------------------------------------------------------------
def prep_inputs(inputs, t_in=T_IN, t_out=T_OUT, n_feat=FPC, n_cores=8, batch=B):
    """Build per-core in_maps from the full problem inputs."""
    x = np.asarray(inputs["x"], np.float32)
    in_maps = []
    # sigma trick scale for i,f,o rows (tanh(x/2)); g rows stay 1.0
    gate_scale = np.concatenate([
        np.full(H, 0.5, np.float32), np.full(H, 0.5, np.float32),
        np.ones(H, np.float32), np.full(H, 0.5, np.float32)])

    for c in range(n_cores):
        m = {}
        # ---------------- encoder (cluster c) ----------------
        xc = x[:batch, :t_in, c * PER:(c + 1) * PER]      # (B, T, 8)
        xe = np.empty((t_in, 9, batch), np.float32)
        xe[:, 0:8, :] = xc.transpose(1, 2, 0)
        xe[:, 8, :] = 1.0
        m["xenc"] = np.ascontiguousarray(xe)

        wih0 = np.asarray(inputs["enc_Wih0"][c], np.float32)   # (4H, PER)
        whh0 = np.asarray(inputs["enc_Whh0"][c], np.float32)   # (4H, H)
        b0 = np.asarray(inputs["enc_bih0"][c] + inputs["enc_bhh0"][c], np.float32)
        wih1 = np.asarray(inputs["enc_Wih1"][c], np.float32)
        whh1 = np.asarray(inputs["enc_Whh1"][c], np.float32)
        b1 = np.asarray(inputs["enc_bih1"][c] + inputs["enc_bhh1"][c], np.float32)

        gs = gate_scale[:, None]
        # L0: x-term lhsT rows = [Wih0^T ; bias], scaled by sigma trick
        l0w = np.zeros((9, G4), np.float32)
        l0w[0:8, :] = (wih0 * gs).T
        l0w[8, :] = b0 * gate_scale
        m["e_l0w"] = l0w
        # L0 h-term: h is doubled -> halve; plus sigma trick
        m["e_l0h"] = np.ascontiguousarray((whh0 * 0.5 * gs).T)
        # L1: input h0 doubled -> halve; sigma trick
        m["e_l1h"] = np.ascontiguousarray((wih1 * 0.5 * gs).T)
        m["e_l1g"] = np.ascontiguousarray((whh1 * 0.5 * gs).T)
        m["e_l1b"] = (b1 * gate_scale)[None, :]

        # ---------------- decoder (features c*8 .. c*8+n_feat) --------------
        ihb = np.zeros((2, n_feat * G4), np.float32)
        whh = np.zeros((H, n_feat * G4), np.float32)
        fc1 = np.zeros((H, n_feat * H), np.float32)
        fc1b = np.zeros((H, n_feat), np.float32)
        Mt = np.zeros((H, n_feat * H), np.float32)
        w1 = np.zeros((H, 2 * n_feat), np.float32)
        Wv = np.zeros((H, n_feat * H), np.float32)
        ow = np.zeros((H, n_feat * H), np.float32)
        fc2 = np.zeros((H, n_feat), np.float32)
        beta = np.zeros((1, n_feat), np.float32)
        x0 = np.ones((2, n_feat * batch), np.float32)

        for j in range(n_feat):
            fi = c * PER + j
            dwih = np.asarray(inputs["dec_Wih"][fi], np.float32)   # (4H, 1)
            dwhh = np.asarray(inputs["dec_Whh"][fi], np.float32)   # (4H, H)
            db = np.asarray(inputs["dec_bih"][fi] + inputs["dec_bhh"][fi], np.float32)
            aw = np.asarray(inputs["attn_in_w"][fi], np.float32)   # (3H, H)
            ab = np.asarray(inputs["attn_in_b"][fi], np.float32)   # (3H,)
            aow = np.asarray(inputs["attn_out_w"][fi], np.float32)  # (H, H)
            aob = np.asarray(inputs["attn_out_b"][fi], np.float32)  # (H,)
            f1w = np.asarray(inputs["fc1_w"][fi], np.float32)      # (H, H)
            f1b = np.asarray(inputs["fc1_b"][fi], np.float32)      # (H,)
            f2w = np.asarray(inputs["fc2_w"][fi], np.float32)      # (1, H)
            f2b = np.asarray(inputs["fc2_b"][fi], np.float32)      # (1,)

            Wq, Wk, Wvv = aw[0:H], aw[H:2 * H], aw[2 * H:3 * H]
            bq, bk, bv = ab[0:H], ab[H:2 * H], ab[2 * H:3 * H]

            ihb[0, j * G4:(j + 1) * G4] = dwih[:, 0] * gate_scale
            ihb[1, j * G4:(j + 1) * G4] = db * gate_scale
            whh[:, j * G4:(j + 1) * G4] = (dwhh * 0.5 * gs).T
            fc1[:, j * H:(j + 1) * H] = (f1w * 0.5).T
            fc1b[:, j] = f1b
            Mt[:, j * H:(j + 1) * H] = (Wk.T @ Wq).T
            w1[:, 2 * j] = SCALE * (Wk.T @ bq)
            w1[:, 2 * j + 1] = w1[:, 2 * j]
            Wv[:, j * H:(j + 1) * H] = Wvv.T
            ow[:, j * H:(j + 1) * H] = aow.T
            fc2[:, j] = f2w[0]
            beta[0, j] = float(f2w[0] @ (aow @ bv + aob) + f2b[0])
            x0[0, j * batch:(j + 1) * batch] = x[:batch, -1, fi]

        const = np.zeros((H, 2 * batch), np.float32)
        const[:, 0:batch] = 1.0
        m.update(d_ihb=ihb, d_whh=whh, d_fc1=fc1, d_fc1b=fc1b, d_M=Mt,
                 d_w1=w1, d_Wv=Wv, d_ow=ow, d_fc2=fc2, d_beta=beta, d_x0=x0,
                 d_const=const)
        in_maps.append(m)
    return in_maps


def assemble_output(results, t_out=T_OUT, n_feat=FPC, batch=B):
    out = np.empty((batch, t_out, len(results) * n_feat), np.float32)
    for c, r in enumerate(results):
        # r["out"]: (t_out, n_feat, B)
        out[:, :, c * n_feat:(c + 1) * n_feat] = r["out"].transpose(2, 0, 1)
    return out


_cached = {}


def kernel(**inputs) -> np.ndarray:
    from concourse.bass_utils import run_bass_kernel_spmd
    key = "full"
    if key not in _cached:
        _cached[key] = build_kernel()
    nc = _cached[key]
    in_maps = prep_inputs(inputs)
    res = run_bass_kernel_spmd(nc, in_maps, core_ids=list(range(8)))
    return assemble_output(res.results)


# revision 28
# speedup vs baseline: 916.8508x; 888.7171x over previous
"""Trainium2 Bass kernel for nn_ClusteringMultiTaskLSTM.

Self-contained: builds + runs an SPMD kernel on 8 NeuronCores.
  - Encoder: one cluster per core (C=8), 2-layer LSTM over T_IN steps,
    transposed layout (dim, B), fp32r (TF32) matmuls.
  - AllReduce of final encoder states across cores (cluster mean).
  - Decoder: 8 per-feature decoders per core (features 8c..8c+7 all belong
    to cluster c), T_OUT steps of LSTM cell + batch-axis attention + fc2.

Math tricks (exact, not approximations):
  - sigmoid(x) = (1 + tanh(x/2))/2 -> all gates use one tanh table set; the
    1/2 input scale is folded into i,f,o weight rows host-side; the output
    affine is folded into fused scalar_tensor_tensor ops via a doubled
    state: S = 2c, hS = 2h, with h-consuming weights pre-halved host-side.
  - attention scores: q.k = qkv^T (Wk^T Wq) qkv + per-key term + per-query
    terms; per-query terms are softmax-invariant and dropped; the per-key
    term becomes the exp() per-partition bias; softmax normalization is
    deferred through the (linear) attn->out_w->fc2 chain and applied as a
    single multiply by 1/colsum on the (1,B) fc2 output.
  - v bias + out_w bias + fc2 bias collapse to one scalar beta per feature.
"""
import sys, traceback
if '/opt/trn_rl_repo' not in sys.path:
    sys.path.insert(0, '/opt/trn_rl_repo')

import numpy as np
import concourse.bass as bass
import concourse.mybir as mybir
from concourse.tile import TileContext
from concourse.vector_clock import ScopedClock

# ----------------------------------------------------------------------------
# Workarounds: the installed walrus accepts only 1 sync-wait per instruction.
# Split excess waits onto NoOp carriers (same engine, program order).
# ----------------------------------------------------------------------------
WAIT_LIMIT = 1


def _make_wait_nop(nc, engine, waits):
    nop = mybir.InstNoOp(name=nc.get_next_instruction_name(), ins=[], outs=[])
    nop.engine = engine
    nop.sync_info = mybir.SyncInfo(on_wait=list(waits), on_update=[])
    return nop


def _split_waits(nc, insts):
    out = []
    for inst in insts:
        si = inst.sync_info
        waits = list(si.on_wait) if (si is not None and si.on_wait) else []
        if len(waits) > WAIT_LIMIT and inst.engine != mybir.EngineType.Unassigned:
            excess, keep = waits[:-WAIT_LIMIT], waits[-WAIT_LIMIT:]
            si.on_wait = keep
            for i in range(0, len(excess), WAIT_LIMIT):
                out.append(_make_wait_nop(nc, inst.engine, excess[i:i + WAIT_LIMIT]))
        out.append(inst)
    return out


if not getattr(TileContext, "_wait_split_patched", False):
    _orig_lower = TileContext._lower_ordered_insts

    def _patched_lower(self, ordered):
        for bb in list(ordered.keys()):
            ordered[bb] = _split_waits(self.nc, ordered[bb])
        return _orig_lower(self, ordered)

    TileContext._lower_ordered_insts = _patched_lower

    def _patched_drain_and_barrier(self, tick_clock, wait_clock):
        drain_inst = self.nc.sync.drain()
        wait_clock.add_sem_waits(drain_inst.ins,
                                 ScopedClock({None: tick_clock.global_clock}))
        si = drain_inst.ins.sync_info
        waits = list(si.on_wait) if si and si.on_wait else []
        if len(waits) > WAIT_LIMIT:
            si.on_wait = waits[:WAIT_LIMIT]
            rest = waits[WAIT_LIMIT:]
            for i in range(0, len(rest), WAIT_LIMIT):
                extra = self.nc.sync.drain()
                esi = extra.ins.sync_info
                if esi is None:
                    extra.ins.sync_info = mybir.SyncInfo(
                        on_wait=rest[i:i + WAIT_LIMIT], on_update=[])
                else:
                    esi.on_wait = rest[i:i + WAIT_LIMIT]
        self.nc.all_engine_barrier()
        assert self.sems is not None
        popped = self.nc._tile_sem_poison_stack.pop()
        assert popped is self._sem_poison
        self.nc.clear_and_free_semaphores(list(self.sems.allocated().values()))
        self.nc.all_engine_barrier()

    TileContext._drain_and_barrier = _patched_drain_and_barrier
    TileContext._wait_split_patched = True


def _install_debug_hook():
    try:
        import libneuronxla
    except ImportError:
        return
    from concourse import bass2jax as _b2j
    _b2j.install_neuronx_cc_hook()
    _cur = libneuronxla.neuronx_cc
    if getattr(_cur, "_is_debug_hook", False):
        return

    def _debug_hook(*args, **kwargs):
        try:
            return _cur(*args, **kwargs)
        except BaseException:
            traceback.print_exc()
            with open('/tmp/hook_log.txt', 'a') as f:
                traceback.print_exc(file=f)
            raise

    _debug_hook._is_debug_hook = True
    libneuronxla.neuronx_cc = _debug_hook
    _b2j.neuronx_cc_hook = _debug_hook


_install_debug_hook()

# ----------------------------------------------------------------------------
# Problem constants
# ----------------------------------------------------------------------------
B, T_IN, T_OUT = 256, 168, 24
F, H, C = 64, 128, 8
PER = F // C          # 8 features per cluster
FPC = F // 8          # 8 features per core (== PER; core c owns cluster c)
G4 = 4 * H            # 512 gate rows
SCALE = 1.0 / np.sqrt(H)

f32 = mybir.dt.float32
f32r = mybir.dt.float32r
AF = mybir.ActivationFunctionType
ALU = mybir.AluOpType


# ----------------------------------------------------------------------------
# Kernel builder (parametric so small configs can be simulated quickly)
# ----------------------------------------------------------------------------
def build_kernel(t_in=T_IN, t_out=T_OUT, n_feat=FPC, n_cores=8, batch=B):
    assert batch % 2 == 0
    Bf = batch               # free-dim batch
    XCH = 21 if t_in % 21 == 0 else t_in   # x DMA chunk length (steps)
    n_ch = t_in // XCH

    nc = bass.Bass()

    # ---- DRAM I/O (per core). float32r tensors still take np.float32 arrays.
    xenc = nc.dram_tensor("xenc", [t_in, 9, Bf], f32r, kind="ExternalInput")
    e_l0w = nc.dram_tensor("e_l0w", [9, G4], f32r, kind="ExternalInput")
    e_l0h = nc.dram_tensor("e_l0h", [H, G4], f32r, kind="ExternalInput")
    e_l1h = nc.dram_tensor("e_l1h", [H, G4], f32r, kind="ExternalInput")
    e_l1g = nc.dram_tensor("e_l1g", [H, G4], f32r, kind="ExternalInput")
    e_l1b = nc.dram_tensor("e_l1b", [1, G4], f32r, kind="ExternalInput")

    d_ihb = nc.dram_tensor("d_ihb", [2, n_feat * G4], f32r, kind="ExternalInput")
    d_whh = nc.dram_tensor("d_whh", [H, n_feat * G4], f32r, kind="ExternalInput")
    d_fc1 = nc.dram_tensor("d_fc1", [H, n_feat * H], f32r, kind="ExternalInput")
    d_fc1b = nc.dram_tensor("d_fc1b", [H, n_feat], f32, kind="ExternalInput")
    d_M = nc.dram_tensor("d_M", [H, n_feat * H], f32r, kind="ExternalInput")
    d_w1 = nc.dram_tensor("d_w1", [H, 2 * n_feat], f32r, kind="ExternalInput")
    d_Wv = nc.dram_tensor("d_Wv", [H, n_feat * H], f32r, kind="ExternalInput")
    d_ow = nc.dram_tensor("d_ow", [H, n_feat * H], f32r, kind="ExternalInput")
    d_fc2 = nc.dram_tensor("d_fc2", [H, n_feat], f32r, kind="ExternalInput")
    d_beta = nc.dram_tensor("d_beta", [1, n_feat], f32, kind="ExternalInput")
    d_x0 = nc.dram_tensor("d_x0", [2, n_feat * Bf], f32r, kind="ExternalInput")
    d_const = nc.dram_tensor("d_const", [H, 2 * Bf], f32r, kind="ExternalInput")

    out_d = nc.dram_tensor("out", [t_out, n_feat, Bf], f32r, kind="ExternalOutput")

    with TileContext(nc) as tc:
        with tc.tile_pool(name="wgt", bufs=1) as wp, \
             tc.tile_pool(name="state", bufs=1) as sp, \
             tc.tile_pool(name="xe", bufs=2) as xp, \
             tc.tile_pool(name="work", bufs=3) as kp, \
             tc.tile_pool(name="gps", bufs=1, space="PSUM") as gps, \
             tc.tile_pool(name="mps", bufs=2, space="PSUM") as mps, \
             tc.tile_pool(name="acc", bufs=1, space="PSUM") as acc, \
             tc.tile_pool(name="dram", bufs=1, space="DRAM") as dp:

            # ---------------- constants / weights into SBUF ----------------
            w_l0w = wp.tile([9, G4], f32r, tag="w_l0w")
            w_l0h = wp.tile([H, G4], f32r, tag="w_l0h")
            w_l1h = wp.tile([H, G4], f32r, tag="w_l1h")
            w_l1g = wp.tile([H, G4], f32r, tag="w_l1g")
            w_l1b = wp.tile([1, G4], f32r, tag="w_l1b")
            nc.sync.dma_start(out=w_l0w[:], in_=e_l0w[:])
            nc.sync.dma_start(out=w_l0h[:], in_=e_l0h[:])
            nc.sync.dma_start(out=w_l1h[:], in_=e_l1h[:])
            nc.sync.dma_start(out=w_l1g[:], in_=e_l1g[:])
            nc.sync.dma_start(out=w_l1b[:], in_=e_l1b[:])

            w_ihb = wp.tile([2, n_feat * G4], f32r, tag="w_ihb")
            w_whh = wp.tile([H, n_feat * G4], f32r, tag="w_whh")
            w_fc1 = wp.tile([H, n_feat * H], f32r, tag="w_fc1")
            w_fc1b = wp.tile([H, n_feat], f32, tag="w_fc1b")
            w_M = wp.tile([H, n_feat * H], f32r, tag="w_M")
            w_w1 = wp.tile([H, 2 * n_feat], f32r, tag="w_w1")
            w_Wv = wp.tile([H, n_feat * H], f32r, tag="w_Wv")
            w_ow = wp.tile([H, n_feat * H], f32r, tag="w_ow")
            w_fc2 = wp.tile([H, n_feat], f32r, tag="w_fc2")
            w_beta = wp.tile([1, n_feat], f32, tag="w_beta")

            ones_row = wp.tile([1, Bf], f32r, tag="ones_row")
            ones_col = wp.tile([H, 1], f32r, tag="ones_col")
            nc.sync.dma_start(out=ones_row[:], in_=d_const[0:1, 0:Bf])
            nc.sync.dma_start(out=ones_col[:], in_=d_const[:, 0:1])

            # ---------------- states ----------------
            h0 = sp.tile([H, Bf], f32r, tag="h0")
            h1 = sp.tile([H, Bf], f32r, tag="h1")
            S0 = sp.tile([H, Bf], f32, tag="S0")
            S1 = sp.tile([H, Bf], f32, tag="S1")
            nc.sync.dma_start(out=h0[:], in_=d_const[:, Bf:2 * Bf])
            nc.sync.dma_start(out=h1[:], in_=d_const[:, Bf:2 * Bf])
            nc.vector.memset(S0[:], 0.0)
            nc.vector.memset(S1[:], 0.0)

            # ============================ ENCODER ============================
            # Generator-based emission: ops of independent cells are
            # interleaved so each engine's in-order stream has independent
            # work to fill dependency gaps (software pipelining).
            uid = [0]

            def tl_(shape, dt_, tag):
                uid[0] += 1
                return kp.tile(shape, dt_, tag=tag, name=f"{tag}_{uid[0]}")

            def drive(gens, window=2):
                from collections import deque
                q = deque(gens)
                active = []
                while q or active:
                    while q and len(active) < window:
                        active.append(q.popleft())
                    for g in list(active):
                        try:
                            next(g)
                        except StopIteration:
                            active.remove(g)

            def lstm_cell_gen(g_tag, emit_mms, S, h):
                uid[0] += 1
                g_ps = gps.tile([H, 4 * Bf], f32, tag=g_tag,
                                name=f"g_{g_tag}_{uid[0]}")
                for _ in emit_mms(g_ps):
                    yield
                T = tl_([H, 4 * Bf], f32, "T")
                nc.scalar.activation(T[:], g_ps[:], AF.Tanh)
                yield
                m1 = tl_([H, Bf], f32, "m1")
                m2 = tl_([H, Bf], f32, "m2")
                nc.vector.scalar_tensor_tensor(
                    m1[:], T[:, Bf:2 * Bf], 1.0, S[:], op0=ALU.add, op1=ALU.mult)
                yield
                nc.vector.scalar_tensor_tensor(
                    m2[:], T[:, 0:Bf], 1.0, T[:, 2 * Bf:3 * Bf],
                    op0=ALU.add, op1=ALU.mult)
                yield
                nc.vector.scalar_tensor_tensor(
                    S[:], m1[:], 0.5, m2[:], op0=ALU.mult, op1=ALU.add)
                yield
                th = tl_([H, Bf], f32, "th")
                nc.scalar.activation(th[:], S[:], AF.Tanh, scale=0.5)
                yield
                nc.vector.scalar_tensor_tensor(
                    h[:], T[:, 3 * Bf:4 * Bf], 1.0, th[:], op0=ALU.add, op1=ALU.mult)
                yield

            xe_tiles = {}

            def get_xt(t):
                ch, tl = t // XCH, t % XCH
                if tl == 0:
                    uid[0] += 1
                    xe = xp.tile([9, XCH * Bf], f32r, tag="xe",
                                 name=f"xe_{uid[0]}")
                    xe_tiles[ch] = xe
                    nc.sync.dma_start(
                        out=xe[:].rearrange("p (t b) -> p t b", b=Bf),
                        in_=xenc[ch * XCH:(ch + 1) * XCH, :, :].rearrange(
                            "t p b -> p t b"))
                return xe_tiles[ch][:, tl * Bf:(tl + 1) * Bf]

            def cell_chain(g_ps, S, h):
                """ACT/DVE tail of one LSTM cell as a generator."""
                T = tl_([H, 4 * Bf], f32, "T")
                nc.scalar.activation(T[:], g_ps[:], AF.Tanh)
                yield
                m1 = tl_([H, Bf], f32, "m1")
                m2 = tl_([H, Bf], f32, "m2")
                nc.vector.scalar_tensor_tensor(
                    m1[:], T[:, Bf:2 * Bf], 1.0, S[:], op0=ALU.add, op1=ALU.mult)
                yield
                nc.vector.scalar_tensor_tensor(
                    m2[:], T[:, 0:Bf], 1.0, T[:, 2 * Bf:3 * Bf],
                    op0=ALU.add, op1=ALU.mult)
                yield
                nc.vector.scalar_tensor_tensor(
                    S[:], m1[:], 0.5, m2[:], op0=ALU.mult, op1=ALU.add)
                yield
                th = tl_([H, Bf], f32, "th")
                nc.scalar.activation(th[:], S[:], AF.Tanh, scale=0.5)
                yield
                nc.vector.scalar_tensor_tensor(
                    h[:], T[:, 3 * Bf:4 * Bf], 1.0, th[:], op0=ALU.add, op1=ALU.mult)
                yield

            # software-pipelined pair: B = L0(t+1) [short path to next pair],
            # A = L1(t). Independent mms first; h0-dependent mms last.
            def enc_pair(t):
                uid[0] += 1
                gA = gps.tile([H, 4 * Bf], f32, tag="g1", name=f"gA_{uid[0]}")
                gB = None
                if t + 1 < t_in:
                    x_t = get_xt(t + 1)
                    uid[0] += 1
                    gB = gps.tile([H, 4 * Bf], f32, tag="g0", name=f"gB_{uid[0]}")
                # phase A: independent matmuls (one accumulation group per
                # tile: start only on the tile's first mm, stop on its last)
                for g in range(4):
                    o = gA[:, g * Bf:(g + 1) * Bf]
                    nc.tensor.matmul(o, w_l1g[:, g * H:(g + 1) * H], h1[:],
                                     start=(g % 2 == 0), stop=False)
                    nc.tensor.matmul(o, w_l1b[:, g * H:(g + 1) * H],
                                     ones_row[:], start=False, stop=False)
                    if gB is not None:
                        nc.tensor.matmul(gB[:, g * Bf:(g + 1) * Bf],
                                         w_l0w[:, g * H:(g + 1) * H], x_t,
                                         start=(g % 2 == 0), stop=False)
                # phase B: h0-dependent matmuls (L0 first: next pair's input)
                if gB is not None:
                    for g in range(4):
                        nc.tensor.matmul(gB[:, g * Bf:(g + 1) * Bf],
                                         w_l0h[:, g * H:(g + 1) * H], h0[:],
                                         start=False, stop=(g % 2 == 1))
                for g in range(4):
                    nc.tensor.matmul(gA[:, g * Bf:(g + 1) * Bf],
                                     w_l1h[:, g * H:(g + 1) * H], h0[:],
                                     start=False, stop=(g % 2 == 1))
                # chains interleaved, L0' ops leading
                chains = [cell_chain(gA, S1, h1)]
                if gB is not None:
                    chains.insert(0, cell_chain(gB, S0, h0))
                drive(chains)

            # prologue: L0(0) alone
            x0t = get_xt(0)
            uid[0] += 1
            g00 = gps.tile([H, 4 * Bf], f32, tag="g0", name=f"g00_{uid[0]}")
            for g in range(4):
                o = g00[:, g * Bf:(g + 1) * Bf]
                nc.tensor.matmul(o, w_l0w[:, g * H:(g + 1) * H], x0t,
                                 start=True, stop=False)
                nc.tensor.matmul(o, w_l0h[:, g * H:(g + 1) * H], h0[:],
                                 start=False, stop=True)
            drive([cell_chain(g00, S0, h0)])
            for t in range(t_in):
                enc_pair(t)

            # decoder weights: emitted after the encoder so their DMA
            # transfers overlap encoder compute on the idle SP queue
            nc.sync.dma_start(out=w_ihb[:], in_=d_ihb[:])
            nc.sync.dma_start(out=w_whh[:], in_=d_whh[:])
            nc.sync.dma_start(out=w_fc1[:], in_=d_fc1[:])
            nc.sync.dma_start(out=w_fc1b[:], in_=d_fc1b[:])
            nc.sync.dma_start(out=w_M[:], in_=d_M[:])
            nc.sync.dma_start(out=w_w1[:], in_=d_w1[:])
            nc.sync.dma_start(out=w_Wv[:], in_=d_Wv[:])
            nc.sync.dma_start(out=w_ow[:], in_=d_ow[:])
            nc.sync.dma_start(out=w_fc2[:], in_=d_fc2[:])
            nc.sync.dma_start(out=w_beta[:], in_=d_beta[:])

            # ===================== ALLREDUCE (cluster mean) ==================
            cc_sb = sp.tile([H, 2 * Bf], f32, tag="cc_sb")
            cc_in = dp.tile([H, 2 * Bf], f32)
            cc_out = dp.tile([H, 2 * Bf], f32)
            nc.sync.dma_start(out=cc_in[:, 0:Bf], in_=h1[:].bitcast(f32))
            nc.sync.dma_start(out=cc_in[:, Bf:2 * Bf], in_=S1[:])
            nc.gpsimd.collective_compute(
                "AllReduce", ALU.add,
                replica_groups=[list(range(n_cores))],
                ins=[cc_in.opt()], outs=[cc_out.opt()])
            nc.sync.dma_start(out=cc_sb[:], in_=cc_out[:])

            # dec states (doubled): h' = 2*(hid+mean)/2 = h1/2 + hsum/16
            # (h1 is 2*hid; hsum is sum of 2*hid over 8 cores)
            dh = sp.tile([H, n_feat * Bf], f32r, tag="dh")
            dS = sp.tile([H, n_feat * Bf], f32, tag="dS")
            mh = kp.tile([H, Bf], f32, tag="mh")
            ms = kp.tile([H, Bf], f32, tag="ms")
            den = 2.0 * n_cores
            nc.vector.tensor_scalar_mul(mh[:], cc_sb[:, 0:Bf], 1.0 / den)
            nc.vector.tensor_scalar_mul(ms[:], cc_sb[:, Bf:2 * Bf], 1.0 / den)
            for f in range(n_feat):
                nc.vector.scalar_tensor_tensor(
                    dh[:, f * Bf:(f + 1) * Bf], h1[:].bitcast(f32), 0.5, mh[:],
                    op0=ALU.mult, op1=ALU.add)
                nc.vector.scalar_tensor_tensor(
                    dS[:, f * Bf:(f + 1) * Bf], S1[:], 0.5, ms[:],
                    op0=ALU.mult, op1=ALU.add)

            # ============================ DECODER ============================
            # feedback buffer at partition 0/1: row 0 = x per feature
            # (overwritten each step), row 1 = ones. Outputs are DMA'd to
            # DRAM once per step before the row is overwritten.
            xb0 = sp.tile([2, n_feat * Bf], f32r, tag="xb0")
            xb1 = sp.tile([2, n_feat * Bf], f32r, tag="xb1")
            XB = [xb0, xb1]
            nc.sync.dma_start(out=xb0[:], in_=d_x0[:])
            nc.sync.dma_start(out=xb1[:], in_=d_x0[:])

            def mtile(shape, tag="m"):
                uid[0] += 1
                return mps.tile(shape, f32, tag=tag, name=f"{tag}_{uid[0]}")

            def featstep(t, f):
                hs = dh[:, f * Bf:(f + 1) * Bf]
                Ss = dS[:, f * Bf:(f + 1) * Bf]
                x_aug = XB[t % 2][0:2, f * Bf:(f + 1) * Bf]
                # ---- LSTM cell (input dim 1 + bias folded into K=2 mm)
                uid[0] += 1
                g_tag = "g0" if (t * n_feat + f) % 2 == 0 else "g1"
                g_ps = gps.tile([H, 4 * Bf], f32, tag=g_tag,
                                name=f"dg_{uid[0]}")
                for g in range(4):
                    o = g_ps[:, g * Bf:(g + 1) * Bf]
                    nc.tensor.matmul(
                        o, w_ihb[:, f * G4 + g * H:f * G4 + (g + 1) * H],
                        x_aug, start=True, stop=False)
                    nc.tensor.matmul(
                        o, w_whh[:, f * G4 + g * H:f * G4 + (g + 1) * H],
                        hs, start=False, stop=True)
                    yield
                T = tl_([H, 4 * Bf], f32, "T")
                nc.scalar.activation(T[:], g_ps[:], AF.Tanh)
                yield
                m1 = tl_([H, Bf], f32, "m1")
                m2 = tl_([H, Bf], f32, "m2")
                nc.vector.scalar_tensor_tensor(
                    m1[:], T[:, Bf:2 * Bf], 1.0, Ss, op0=ALU.add, op1=ALU.mult)
                yield
                nc.vector.scalar_tensor_tensor(
                    m2[:], T[:, 0:Bf], 1.0, T[:, 2 * Bf:3 * Bf],
                    op0=ALU.add, op1=ALU.mult)
                yield
                nc.vector.scalar_tensor_tensor(
                    Ss, m1[:], 0.5, m2[:], op0=ALU.mult, op1=ALU.add)
                yield
                th = tl_([H, Bf], f32, "th")
                nc.scalar.activation(th[:], Ss, AF.Tanh, scale=0.5)
                yield
                nc.vector.scalar_tensor_tensor(
                    hs, T[:, 3 * Bf:4 * Bf], 1.0, th[:], op0=ALU.add, op1=ALU.mult)
                yield

                # ---- qkv = lrelu(fc1 @ h + b)   (fc1 pre-halved for 2h)
                q_ps = mtile([H, Bf])
                nc.tensor.matmul(q_ps[:], w_fc1[:, f * H:(f + 1) * H], hs,
                                 start=True, stop=True)
                yield
                qkv = tl_([H, Bf], f32r, "qkv")
                nc.scalar.activation(qkv[:], q_ps[:], AF.Lrelu,
                                     bias=w_fc1b[:, f:f + 1], alpha=0.01)
                yield

                # ---- z = (Wk^T Wq) @ qkv ; per-key bias = w1 . qkv
                z_ps = mtile([H, Bf])
                nc.tensor.matmul(z_ps[:], w_M[:, f * H:(f + 1) * H], qkv[:],
                                 start=True, stop=True)
                yield
                z = tl_([H, Bf], f32r, "z")
                nc.vector.tensor_copy(z[:], z_ps[:])
                yield
                sb_ps = mtile([H, 4])
                for k in range(2):
                    nc.tensor.matmul(sb_ps[:, 2 * k:2 * k + 2],
                                     qkv[:, k * H:(k + 1) * H],
                                     w_w1[:, 2 * f:2 * f + 2], start=True, stop=True)
                yield
                sbias = tl_([H, 4], f32, "sbias")
                nc.vector.tensor_copy(sbias[:], sb_ps[:])
                yield

                # ---- v = qkv^T @ Wv (per key-chunk), in (B,H) layout
                v_ps = mtile([H, 2 * H])
                for k in range(2):
                    nc.tensor.matmul(v_ps[:, k * H:(k + 1) * H],
                                     qkv[:, k * H:(k + 1) * H],
                                     w_Wv[:, f * H:(f + 1) * H],
                                     start=True, stop=True)
                    yield
                v = tl_([H, 2 * H], f32r, "v")
                nc.vector.tensor_copy(v[:], v_ps[:])
                yield

                # ---- scores_T (key-part, query-free) + exp
                uid[0] += 1
                sc_ps = acc.tile([H, 2 * Bf], f32, tag="sc", name=f"sc_{uid[0]}")
                for k in range(2):
                    nc.tensor.matmul(sc_ps[:, k * Bf:(k + 1) * Bf],
                                     qkv[:, k * H:(k + 1) * H], z[:],
                                     start=True, stop=True)
                    yield
                expT = tl_([H, 2 * Bf], f32r, "expT")
                for k in range(2):
                    nc.scalar.activation(expT[:, k * Bf:(k + 1) * Bf],
                                         sc_ps[:, k * Bf:(k + 1) * Bf],
                                         AF.Exp, bias=sbias[:, 2 * k:2 * k + 1],
                                         scale=SCALE)
                    yield

                # ---- colsum (1,B) and unnormalized ao_T = v^T @ expT
                uid[0] += 1
                aocs = acc.tile([H, 2 * Bf], f32, tag="acc", name=f"acc_{uid[0]}")
                ao_ps = aocs[:, 0:Bf]
                cs_ps = aocs[0:1, Bf:Bf + Bf]
                for k in range(2):
                    nc.tensor.matmul(cs_ps, ones_col[:],
                                     expT[:, k * Bf:(k + 1) * Bf],
                                     start=(k == 0), stop=(k == 1))
                yield
                for k in range(2):
                    nc.tensor.matmul(ao_ps, v[:, k * H:(k + 1) * H],
                                     expT[:, k * Bf:(k + 1) * Bf],
                                     start=(k == 0), stop=(k == 1))
                    yield
                recip = tl_([1, Bf], f32, "recip")
                nc.vector.reciprocal(recip[:], cs_ps)
                yield
                ao = tl_([H, Bf], f32r, "ao")
                nc.vector.tensor_copy(ao[:], ao_ps)
                yield

                # ---- out_w @ ao ; fc2 ; normalize ; lrelu(+beta)
                a2_ps = mtile([H, Bf])
                nc.tensor.matmul(a2_ps[:], w_ow[:, f * H:(f + 1) * H], ao[:],
                                 start=True, stop=True)
                yield
                ao2 = tl_([H, Bf], f32r, "ao2")
                nc.vector.tensor_copy(ao2[:], a2_ps[:])
                yield
                y_ps = mtile([1, Bf])
                nc.tensor.matmul(y_ps[:], w_fc2[:, f:f + 1], ao2[:],
                                 start=True, stop=True)
                yield
                yn = tl_([1, Bf], f32, "yn")
                nc.vector.tensor_tensor(yn[:], y_ps[:], recip[:], op=ALU.mult)
                yield
                nc.scalar.activation(XB[(t + 1) % 2][0:1, f * Bf:(f + 1) * Bf],
                                     yn[:], AF.Lrelu, bias=w_beta[:, f:f + 1],
                                     alpha=0.01)
                yield

            done_count = {}

            def featstep_wrapped(t, f):
                for _ in featstep(t, f):
                    yield
                # last-completing featstep of step t emits the step-output
                # DMA: all step-t x-writes are emitted (deps captured), no
                # step-t+1 x-write is emitted yet (WAR ordered after).
                done_count[t] = done_count.get(t, 0) + 1
                if done_count[t] == n_feat:
                    nc.sync.dma_start(
                        out=out_d[t].rearrange("f b -> (f b)"),
                        in_=XB[(t + 1) % 2][0:1, :])

            dec_stream = []
            for t in range(t_out):
                for f in range(n_feat):
                    dec_stream.append(featstep_wrapped(t, f))
            import os
            drive(dec_stream, window=min(int(os.environ.get('DEC_WIN', '3')), n_feat))

    return nc


# ----------------------------------------------------------------------------
# Host-side weight prep
# ----------------------------------------------------------------------------
def prep_inputs(inputs, t_in=T_IN, t_out=T_OUT, n_feat=FPC, n_cores=8, batch=B):
    """Build per-core in_maps from the full problem inputs."""
    x = np.asarray(inputs["x"], np.float32)
    in_maps = []
    # sigma trick scale for i,f,o rows (tanh(x/2)); g rows stay 1.0
    gate_scale = np.concatenate([
        np.full(H, 0.5, np.float32), np.full(H, 0.5, np.float32),
        np.ones(H, np.float32), np.full(H, 0.5, np.float32)])

    for c in range(n_cores):
        m = {}
        # ---------------- encoder (cluster c) ----------------
        xc = x[:batch, :t_in, c * PER:(c + 1) * PER]      # (B, T, 8)
        xe = np.empty((t_in, 9, batch), np.float32)
        xe[:, 0:8, :] = xc.transpose(1, 2, 0)
        xe[:, 8, :] = 1.0
        m["xenc"] = np.ascontiguousarray(xe)

        wih0 = np.asarray(inputs["enc_Wih0"][c], np.float32)   # (4H, PER)
        whh0 = np.asarray(inputs["enc_Whh0"][c], np.float32)   # (4H, H)
        b0 = np.asarray(inputs["enc_bih0"][c] + inputs["enc_bhh0"][c], np.float32)
        wih1 = np.asarray(inputs["enc_Wih1"][c], np.float32)
        whh1 = np.asarray(inputs["enc_Whh1"][c], np.float32)
        b1 = np.asarray(inputs["enc_bih1"][c] + inputs["enc_bhh1"][c], np.float32)

        gs = gate_scale[:, None]
        # L0: x-term lhsT rows = [Wih0^T ; bias], scaled by sigma trick
        l0w = np.zeros((9, G4), np.float32)
        l0w[0:8, :] = (wih0 * gs).T
        l0w[8, :] = b0 * gate_scale
        m["e_l0w"] = l0w
        # L0 h-term: h is doubled -> halve; plus sigma trick
        m["e_l0h"] = np.ascontiguousarray((whh0 * 0.5 * gs).T)
        # L1: input h0 doubled -> halve; sigma trick
        m["e_l1h"] = np.ascontiguousarray((wih1 * 0.5 * gs).T)
        m["e_l1g"] = np.ascontiguousarray((whh1 * 0.5 * gs).T)
        m["e_l1b"] = (b1 * gate_scale)[None, :]

        # ---------------- decoder (features c*8 .. c*8+n_feat) --------------
        ihb = np.zeros((2, n_feat * G4), np.float32)
        whh = np.zeros((H, n_feat * G4), np.float32)
        fc1 = np.zeros((H, n_feat * H), np.float32)
        fc1b = np.zeros((H, n_feat), np.float32)
        Mt = np.zeros((H, n_feat * H), np.float32)
        w1 = np.zeros((H, 2 * n_feat), np.float32)
        Wv = np.zeros((H, n_feat * H), np.float32)
        ow = np.zeros((H, n_feat * H), np.float32)
        fc2 = np.zeros((H, n_feat), np.float32)
        beta = np.zeros((1, n_feat), np.float32)
        x0 = np.ones((2, n_feat * batch), np.float32)

        for j in range(n_feat):
            fi = c * PER + j
            dwih = np.asarray(inputs["dec_Wih"][fi], np.float32)   # (4H, 1)
            dwhh = np.asarray(inputs["dec_Whh"][fi], np.float32)   # (4H, H)
            db = np.asarray(inputs["dec_bih"][fi] + inputs["dec_bhh"][fi], np.float32)
            aw = np.asarray(inputs["attn_in_w"][fi], np.float32)   # (3H, H)
            ab = np.asarray(inputs["attn_in_b"][fi], np.float32)   # (3H,)
            aow = np.asarray(inputs["attn_out_w"][fi], np.float32)  # (H, H)
            aob = np.asarray(inputs["attn_out_b"][fi], np.float32)  # (H,)
            f1w = np.asarray(inputs["fc1_w"][fi], np.float32)      # (H, H)
            f1b = np.asarray(inputs["fc1_b"][fi], np.float32)      # (H,)
            f2w = np.asarray(inputs["fc2_w"][fi], np.float32)      # (1, H)
            f2b = np.asarray(inputs["fc2_b"][fi], np.float32)      # (1,)

            Wq, Wk, Wvv = aw[0:H], aw[H:2 * H], aw[2 * H:3 * H]
            bq, bk, bv = ab[0:H], ab[H:2 * H], ab[2 * H:3 * H]

            ihb[0, j * G4:(j + 1) * G4] = dwih[:, 0] * gate_scale
            ihb[1, j * G4:(j + 1) * G4] = db * gate_scale
            whh[:, j * G4:(j + 1) * G4] = (dwhh * 0.5 * gs).T
            fc1[:, j * H:(j + 1) * H] = (f1w * 0.5).T
            fc1b[:, j] = f1b
            Mt[:, j * H:(j + 1) * H] = (Wk.T @ Wq).T
            w1[:, 2 * j] = SCALE * (Wk.T @ bq)
            w1[:, 2 * j + 1] = w1[:, 2 * j]
            Wv[:, j * H:(j + 1) * H] = Wvv.T
            ow[:, j * H:(j + 1) * H] = aow.T
            fc2[:, j] = f2w[0]
            beta[0, j] = float(f2w[0] @ (aow @ bv + aob) + f2b[0])
            x0[0, j * batch:(j + 1) * batch] = x[:batch, -1, fi]

        const = np.zeros((H, 2 * batch), np.float32)
        const[:, 0:batch] = 1.0
        m.update(d_ihb=ihb, d_whh=whh, d_fc1=fc1, d_fc1b=fc1b, d_M=Mt,
                 d_w1=w1, d_Wv=Wv, d_ow=ow, d_fc2=fc2, d_beta=beta, d_x0=x0,
                 d_const=const)
        in_maps.append(m)
    return in_maps


def assemble_output(results, t_out=T_OUT, n_feat=FPC, batch=B):
    out = np.empty((batch, t_out, len(results) * n_feat), np.float32)
    for c, r in enumerate(results):
        # r["out"]: (t_out, n_feat, B)
        out[:, :, c * n_feat:(c + 1) * n_feat] = r["out"].transpose(2, 0, 1)
    return out


_cached = {}


def kernel(**inputs) -> np.ndarray:
    from concourse.bass_utils import run_bass_kernel_spmd
    key = "full"
    if key not in _cached:
        _cached[key] = build_kernel()
    nc = _cached[key]
    in_maps = prep_inputs(inputs)
    res = run_bass_kernel_spmd(nc, in_maps, core_ids=list(range(8)))
    return assemble_output(res.results)
